# revision 1
# baseline (speedup 1.0000x reference)
"""Trainium2 Bass kernel for a DAT-style transformer block (sparse_attention).

kernel(**inputs) takes FULL unsharded inputs (B=64), shards the batch across
8 NeuronCores (8 per core, pure data parallel — no collectives), runs one SPMD
Bass/Tile program, returns the FULL [64, 196, 768] float32 output.

Per-core pipeline (8 local batches):
  ST1 LN1 + PE transposes -> xnT (f32, spilled to DRAM) + xnTb (bf16)
  ST2 q = Wq@xnT + bq  -> qp, padded [16,16] spatial layout (bf16)
  ST3 depthwise 3x3 conv via 9 accumulating diag-matmuls; transpose -> ocT
  ST4 offset head: group-LN -> GELU -> proj -> tanh -> pixel coords ->
      4 bilinear tap indices (always in-bounds for |offset|<=0.5px) + weights;
      indices wrapped into the GPSIMD gather layout via selection matmuls
  ST5 gpsimd.ap_gather from xnT image + PE-broadcast bilinear weights -> sampled
  ST6 k = Wk@sampled + bk; vT = sampled^T @ Wv^T
  ST7 attention per head (S -> exp(+rowsum) -> norm -> P^T -> @v), o-proj,
      residual 1, LN2 (x2 spilled to DRAM), transposes -> xn2T
  ST9 MLP (24x fused h1 -> GELU -> accumulate m2), bias, transpose, residual 2
All matmuls bf16 with fp32 PSUM accumulation; stats/softmax/residuals fp32.
"""

import numpy as np
import ml_dtypes

import concourse.bass as bass
import concourse.mybir as mybir
import concourse.tile as tile
from concourse import library_config
from concourse.bass_utils import run_bass_kernel_spmd

FP32 = mybir.dt.float32
BF16 = mybir.dt.bfloat16
I16 = mybir.dt.int16
AF = mybir.ActivationFunctionType
ALU = mybir.AluOpType

B = 64
NCORES = 8
BL = 8
N = 196
C = 768
NCH = 6
HEADS = 12
HD = 64
G = 8
CG = 96
MLPD = 3072
MMCH = 24
HH = 14
NCK = [(0, 128), (128, 68)]
MCK = [(0, 98), (98, 98)]
EPS = 1e-6
OFF_EPS = 1e-5
NPAD = 208
NTAP = 4
QPW = 290


def _f32(x):
    return np.ascontiguousarray(np.asarray(x), dtype=np.float32)


def _bf16(x):
    return np.ascontiguousarray(
        np.asarray(x, dtype=np.float32).astype(ml_dtypes.bfloat16))


def build_host_consts(inp):
    h = {}
    h['WqT'] = _bf16(np.asarray(inp['Wq'], np.float32).T)
    h['WkT'] = _bf16(np.asarray(inp['Wk'], np.float32).T)
    h['WvT'] = _bf16(np.asarray(inp['Wv'], np.float32).T)
    h['WoT'] = _bf16(np.asarray(inp['Wo'], np.float32).T)
    h['W1T'] = _bf16(np.asarray(inp['W1'], np.float32).T)
    h['W2T'] = _bf16(np.asarray(inp['W2'], np.float32).T)

    h['bq'] = _f32(np.asarray(inp['bq']).reshape(NCH, 128).T)
    h['bk'] = _f32(np.asarray(inp['bk']).reshape(NCH, 128).T)
    h['bo'] = _f32(np.asarray(inp['bo']).reshape(NCH, 128).T)
    h['b1'] = _f32(np.asarray(inp['b1']).reshape(MMCH, 128).T)
    h['b2'] = _f32(np.asarray(inp['b2']).reshape(NCH, 128).T)

    dw = np.asarray(inp['off_dw_w'], np.float32).reshape(CG, 9)
    dwg = np.tile(dw, (G, 1))
    diag = np.zeros((9, NCH, 128, 128), np.float32)
    for t in range(9):
        for cc in range(NCH):
            np.fill_diagonal(diag[t, cc], dwg[cc * 128:(cc + 1) * 128, t])
    h['dwdiag'] = _bf16(diag)
    h['dwb'] = _f32(np.tile(np.asarray(inp['off_dw_b'], np.float32), G)
                    .reshape(NCH, 128).T)

    e8 = np.zeros((G, C), np.float32)
    for c in range(C):
        e8[c // CG, c] = 1.0
    h['E8'] = _bf16(e8)

    sel = np.zeros((2, 128, 13 * 128), np.float32)
    for i, (off, nsz) in enumerate(NCK):
        for nl in range(nsz):
            f, p16 = divmod(off + nl, 16)
            for band in range(8):
                sel[i, nl, f * 128 + band * 16 + p16] = 1.0
    h['SelW'] = _bf16(sel)

    ii = np.arange(HH, dtype=np.float32)
    h['refy'] = _f32(np.repeat((ii + 0.5) * 13.0 / 14.0, HH))
    h['refx'] = _f32(np.tile((ii + 0.5) * 13.0 / 14.0, HH))
    h['rowi'] = _f32(np.repeat(ii, HH))
    h['colj'] = _f32(np.tile(ii, HH))

    pw = np.asarray(inp['off_proj_w'], np.float32)
    h['wyv'] = _bf16(np.tile(pw[0], G))
    h['wxv'] = _bf16(np.tile(pw[1], G))

    h['id32'] = _f32(np.eye(128, dtype=np.float32))
    h['id16'] = _bf16(np.eye(128, dtype=np.float32))

    for nm, gk, bk_ in (('ln1', 'ln1_g', 'ln1_b'), ('ln2', 'ln2_g', 'ln2_b')):
        g = np.asarray(inp[gk], np.float32)
        bb = np.asarray(inp[bk_], np.float32)
        h[nm + '_trivial'] = bool(np.all(g == 1.0) and np.all(bb == 0.0))
        h[nm + '_g'] = _f32(g)
        h[nm + '_b'] = _f32(bb)
    og = np.tile(np.asarray(inp['off_ln_g'], np.float32), G)
    ob = np.tile(np.asarray(inp['off_ln_b'], np.float32), G)
    h['offln_trivial'] = bool(np.all(og == 1.0) and np.all(ob == 0.0))
    h['offln_g'] = _f32(og)
    h['offln_b'] = _f32(ob)
    bv = np.asarray(inp['bv'], np.float32)
    h['bv_trivial'] = bool(np.all(bv == 0.0))
    h['bv'] = _f32(np.tile(bv.reshape(1, C), (128, 1)))
    return h


def _free_bcast(t_ap, inner):
    """View [P, F] AP as [P, F, inner] with a stride-0 inner dim."""
    return bass.AP(tensor=t_ap.tensor, offset=t_ap.offset,
                   ap=list(t_ap.ap) + [[0, inner]])


def _dram_bcast(src_ap, rows):
    return bass.AP(tensor=src_ap.tensor, offset=src_ap.offset,
                   ap=[[0, rows]] + list(src_ap.ap))


def emit(nc, tc, d, out_dram, x2_dram, xnT_dram, h):
    x_in = d['x_shard']

    with (
        tc.tile_pool(name='cw', bufs=1) as cw,
        tc.tile_pool(name='p_xn2T', bufs=1) as p_xn2T,
        tc.tile_pool(name='p_sm', bufs=4) as sm,
    ):
        # ---- always-resident constants --------------------------------
        WoT = [cw.tile([128, C], BF16, tag=f'wo{k}', name=f'wo{k}')
               for k in range(NCH)]
        for k in range(NCH):
            nc.sync.dma_start(out=WoT[k][:], in_=d['WoT'][k * 128:(k + 1) * 128, :])
        E8 = cw.tile([G, C], BF16, tag='e8', name='e8')
        nc.sync.dma_start(out=E8[:], in_=d['E8'][:])
        id32 = cw.tile([128, 128], FP32, tag='id32', name='id32')
        id16 = cw.tile([128, 128], BF16, tag='id16', name='id16')
        nc.sync.dma_start(out=id32[:], in_=d['id32'][:])
        nc.sync.dma_start(out=id16[:], in_=d['id16'][:])
        bias_t = {}
        for nm, cols in (('bq', NCH), ('bk', NCH), ('bo', NCH), ('b1', MMCH),
                         ('b2', NCH), ('dwb', NCH)):
            bias_t[nm] = cw.tile([128, cols], FP32, tag='bias_' + nm,
                                 name='bias_' + nm)
            nc.sync.dma_start(out=bias_t[nm][:], in_=d[nm][:])
        refy_t, refx_t, rowi_t, colj_t = [], [], [], []
        for i, (off, nsz) in enumerate(NCK):
            for nm, lst in (('refy', refy_t), ('refx', refx_t),
                            ('rowi', rowi_t), ('colj', colj_t)):
                tt = cw.tile([nsz, 1], FP32, tag=f'{nm}{i}', name=f'{nm}{i}')
                nc.sync.dma_start(
                    out=tt[:],
                    in_=d[nm][off:off + nsz].rearrange('(n one) -> n one',
                                                       one=1))
                lst.append(tt)
        wyb = cw.tile([128, C], BF16, tag='wyb', name='wyb')
        wxb = cw.tile([128, C], BF16, tag='wxb', name='wxb')
        nc.sync.dma_start(out=wyb[:], in_=_dram_bcast(d['wyv'][:], 128))
        nc.sync.dma_start(out=wxb[:], in_=_dram_bcast(d['wxv'][:], 128))
        eps_t = cw.tile([128, 1], FP32, tag='eps', name='eps')
        nc.vector.memset(eps_t[:], EPS)
        oeps_t = cw.tile([128, 1], FP32, tag='oeps', name='oeps')
        nc.vector.memset(oeps_t[:], OFF_EPS)
        gbt = {}
        for nm in ('ln1', 'ln2', 'offln'):
            if not h[nm + '_trivial']:
                g_ = cw.tile([128, C], FP32, tag=nm + 'g', name=nm + 'g')
                b_ = cw.tile([128, C], FP32, tag=nm + 'b', name=nm + 'b')
                nc.sync.dma_start(out=g_[:], in_=_dram_bcast(d[nm + '_g'][:], 128))
                nc.sync.dma_start(out=b_[:], in_=_dram_bcast(d[nm + '_b'][:], 128))
                gbt[nm] = (g_, b_)
        bv_t = None
        if not h['bv_trivial']:
            bv_t = cw.tile([128, C], FP32, tag='bvt', name='bvt')
            nc.sync.dma_start(out=bv_t[:], in_=d['bv'][:])

        def ln_norm(xf, nsz, out_ap, gbk, tmp_pool):
            st = sm.tile([128, 3, 6], FP32, tag='ln_st', name='ln_st')
            for s in range(3):
                nc.vector.bn_stats(out=st[:nsz, s, :],
                                   in_=xf[:nsz, s * 256:(s + 1) * 256])
            mv = sm.tile([128, 2], FP32, tag='ln_mv', name='ln_mv')
            nc.vector.bn_aggr(out=mv[:nsz], in_=st[:nsz])
            std = sm.tile([128, 1], FP32, tag='ln_std', name='ln_std')
            nc.scalar.activation(out=std[:nsz], in_=mv[:nsz, 1:2], func=AF.Sqrt,
                                 bias=eps_t[:nsz], scale=1.0)
            rstd = sm.tile([128, 1], FP32, tag='ln_rstd', name='ln_rstd')
            nc.vector.reciprocal(out=rstd[:nsz], in_=std[:nsz])
            nmr = sm.tile([128, 1], FP32, tag='ln_nmr', name='ln_nmr')
            nc.vector.tensor_scalar(out=nmr[:nsz], in0=mv[:nsz, 0:1],
                                    scalar1=rstd[:nsz], scalar2=-1.0,
                                    op0=ALU.mult, op1=ALU.mult)
            if gbk is None:
                nc.scalar.activation(out=out_ap, in_=xf[:nsz], func=AF.Identity,
                                     bias=nmr[:nsz], scale=rstd[:nsz])
            else:
                gt, bt = gbk
                tmp = tmp_pool.tile([128, C], FP32, tag='ln_tmp', name='ln_tmp')
                nc.scalar.activation(out=tmp[:nsz], in_=xf[:nsz], func=AF.Identity,
                                     bias=nmr[:nsz], scale=rstd[:nsz])
                nc.vector.tensor_mul(out=tmp[:nsz], in0=tmp[:nsz], in1=gt[:nsz])
                nc.vector.tensor_add(out=out_ap, in0=tmp[:nsz], in1=bt[:nsz])

        xn2T = [p_xn2T.tile([128, BL, N], BF16, tag=f'x2T{k}', name=f'x2T{k}')
                for k in range(NCH)]

        with tc.tile_pool(name='p_qp', bufs=1) as p_qp:
            q_pl = [p_qp.tile([128, BL, N], BF16, tag=f'qpl{k}', name=f'qpl{k}')
                    for k in range(NCH)]
            with tc.tile_pool(name='p_smp', bufs=1) as p_smp:
                sampled = [p_smp.tile([128, BL, N], BF16, tag=f'smp{k}',
                                      name=f'smp{k}') for k in range(NCH)]
                with tc.tile_pool(name='p_qpad', bufs=1) as p_qpad:
                    qp = [p_qpad.tile([128, BL * QPW], BF16, tag=f'qp{k}',
                                      name=f'qp{k}') for k in range(NCH)]
                    for k in range(NCH):
                        nc.vector.memset(qp[k][:], 0.0)
                    with (
                        tc.tile_pool(name='p_st12', bufs=1) as p12,
                        tc.tile_pool(name='t_st12', bufs=3) as tp1,
                        tc.tile_pool(name='tb_st12', bufs=2) as tb1,
                        tc.tile_pool(name='ps_st12', bufs=3, space='PSUM') as pp1,
                        tc.tile_pool(name='ps_st12b', bufs=2, space='PSUM') as pp1b,
                    ):
                        WqT = [p12.tile([128, C], BF16, tag=f'wq{k}', name=f'wq{k}')
                               for k in range(NCH)]
                        for k in range(NCH):
                            nc.sync.dma_start(out=WqT[k][:],
                                              in_=d['WqT'][k * 128:(k + 1) * 128, :])
                        xnTb = [p12.tile([128, BL, N], BF16, tag=f'xnTb{k}',
                                         name=f'xnTb{k}') for k in range(NCH)]

                        for b in range(BL):
                            xnt_tmp = tp1.tile([128, NCH, N], FP32, tag='st1_xnT',
                                               name='st1_xnT')
                            for i, (off, nsz) in enumerate(NCK):
                                xf = tb1.tile([128, C], FP32, tag='st1_x', name='st1_x')
                                nc.sync.dma_start(out=xf[:nsz],
                                                  in_=x_in[b, off:off + nsz, :])
                                xn = tb1.tile([128, C], FP32, tag='st1_xn', name='st1_xn')
                                ln_norm(xf, nsz, xn[:nsz], gbt.get('ln1'), tb1)
                                for cc in range(NCH):
                                    pt = pp1.tile([128, 128], FP32, tag='tp128',
                                                  name='st1_ps')
                                    nc.tensor.transpose(
                                        pt[:, :nsz], xn[:nsz, cc * 128:(cc + 1) * 128],
                                        id32[:nsz, :nsz])
                                    nc.scalar.activation(
                                        out=xnt_tmp[:, cc, off:off + nsz],
                                        in_=pt[:, :nsz], func=AF.Identity)
                                    nc.vector.tensor_copy(
                                        out=xnTb[cc][:, b, off:off + nsz],
                                        in_=pt[:, :nsz])
                            nc.sync.dma_start(out=xnT_dram[b], in_=xnt_tmp[:])

                        for bp in range(0, BL, 2):
                            for oc in range(NCH):
                                q_ps = pp1b.tile([128, 392], FP32, tag='acc392',
                                                 name='st2_ps')
                                for kc in range(NCH):
                                    nc.tensor.matmul(
                                        q_ps[:], WqT[kc][:, oc * 128:(oc + 1) * 128],
                                        xnTb[kc][:, bp:bp + 2, :],
                                        start=(kc == 0), stop=(kc == NCH - 1))
                                for bi, b in enumerate((bp, bp + 1)):
                                    base = qp[oc][:, b * QPW + 17:b * QPW + 18]
                                    outap = bass.AP(tensor=base.tensor,
                                                    offset=base.offset,
                                                    ap=[base.ap[0], [16, 14], [1, 14]])
                                    nc.scalar.activation(
                                        out=outap,
                                        in_=q_ps[:, bi * N:(bi + 1) * N],
                                        func=AF.Identity,
                                        bias=bias_t['bq'][:, oc:oc + 1])
                                nc.vector.tensor_copy(
                                    out=q_pl[oc][:, bp:bp + 2, :],
                                    in_=q_ps[:].rearrange('p (b n) -> p b n', b=2))

                    # ============ ST3..ST5 (pair loop), then ST6, ST7 ===========
                    with (
                        tc.tile_pool(name='p_cs', bufs=1) as pcs,
                        tc.tile_pool(name='t_cs', bufs=3) as tp3,
                        tc.tile_pool(name='ps_cs', bufs=2, space='PSUM') as pp3,
                        tc.tile_pool(name='ps_cs2', bufs=2, space='PSUM') as pp3b,
                    ):
                        dwdiag = pcs.tile([128, 9 * NCH * 128], BF16, tag='dwdiag',
                                          name='dwdiag')
                        nc.sync.dma_start(
                            out=dwdiag[:].rearrange('p (t c m) -> p t c m',
                                                    t=9, c=NCH),
                            in_=d['dwdiag'][:].rearrange('t c p m -> p t c m'))
                        SelW = [pcs.tile([128, 13 * 128], BF16, tag=f'sel{i}',
                                         name=f'sel{i}') for i in range(2)]
                        for i in range(2):
                            nc.sync.dma_start(out=SelW[i][:], in_=d['SelW'][i])

                        for bp in range(0, BL, 2):
                            _conv_offset_gather(
                                nc, tc, d, h, bp, qp, dwdiag, SelW, E8, id16,
                                bias_t, refy_t, refx_t, rowi_t, colj_t,
                                wyb, wxb, oeps_t, gbt,
                                sampled, xnT_dram, tp3, sm, pp3, pp3b)

                with (
                    tc.tile_pool(name='p_kv', bufs=1) as pkv,
                    tc.tile_pool(name='t_67', bufs=3) as tp7,
                    tc.tile_pool(name='tb_67', bufs=2) as tb7,
                    tc.tile_pool(name='ps_67', bufs=2, space='PSUM') as pp7,
                    tc.tile_pool(name='ps_67b', bufs=2, space='PSUM') as pp7b,
                ):
                    WkT = [pkv.tile([128, C], BF16, tag=f'wk{k}', name=f'wk{k}')
                           for k in range(NCH)]
                    WvT = [pkv.tile([128, C], BF16, tag=f'wv{k}', name=f'wv{k}')
                           for k in range(NCH)]
                    for k in range(NCH):
                        nc.sync.dma_start(out=WkT[k][:],
                                          in_=d['WkT'][k * 128:(k + 1) * 128, :])
                        nc.sync.dma_start(out=WvT[k][:],
                                          in_=d['WvT'][k * 128:(k + 1) * 128, :])
                    k_all = [pkv.tile([128, BL, N], BF16, tag=f'kk{k}',
                                      name=f'kk{k}') for k in range(NCH)]
                    vT_all = pkv.tile([128, BL, 2, C], BF16, tag='vT', name='vT')

                    # ---------------- ST6 --------------------------------
                    for bp in range(0, BL, 2):
                        for oc in range(NCH):
                            k_ps = pp7.tile([128, 392], FP32, tag='acc392',
                                            name='st6_kps')
                            for kc in range(NCH):
                                nc.tensor.matmul(
                                    k_ps[:], WkT[kc][:, oc * 128:(oc + 1) * 128],
                                    sampled[kc][:, bp:bp + 2, :],
                                    start=(kc == 0), stop=(kc == NCH - 1))
                            nc.scalar.activation(
                                out=k_all[oc][:, bp:bp + 2, :],
                                in_=k_ps[:].rearrange('p (b n) -> p b n', b=2),
                                func=AF.Identity,
                                bias=bias_t['bk'][:, oc:oc + 1])
                    for b in range(BL):
                        for i, (off, nsz) in enumerate(NCK):
                            for half in range(2):
                                v_ps = pp7.tile([128, 384], FP32, tag='acc392',
                                                name='st6_vps')
                                for kc in range(NCH):
                                    nc.tensor.matmul(
                                        v_ps[:nsz],
                                        sampled[kc][:, b, off:off + nsz],
                                        WvT[kc][:, half * 384:(half + 1) * 384],
                                        start=(kc == 0), stop=(kc == NCH - 1))
                                dst = vT_all[:nsz, b, i,
                                             half * 384:(half + 1) * 384]
                                if bv_t is None:
                                    nc.vector.tensor_copy(out=dst, in_=v_ps[:nsz])
                                else:
                                    nc.vector.tensor_add(
                                        out=dst, in0=v_ps[:nsz],
                                        in1=bv_t[:nsz,
                                                 half * 384:(half + 1) * 384])

                    # ---------------- ST7 --------------------------------
                    for bp in range(0, BL, 2):
                        aop = tp7.tile([128, NCH, 2, N], BF16, tag='st7_ao',
                                       name='st7_ao')
                        for bi, b in enumerate((bp, bp + 1)):
                            for hp in range(NCH):
                                o_ps = pp7b.tile([128, N], FP32, tag='st7_ops',
                                                 name='st7_ops')
                                for hh in range(2):
                                    hd = hp * 2 + hh
                                    p0 = (hd % 2) * 64
                                    PT = [tp7.tile([128, N], BF16, tag='st7_pt',
                                                   name='st7_pt')
                                          for _ in range(2)]
                                    for mi, (moff, msz) in enumerate(MCK):
                                        s_ps = pp7b.tile([98, N], FP32,
                                                         tag='st7_sps',
                                                         name='st7_sps')
                                        nc.tensor.matmul(
                                            s_ps[:],
                                            q_pl[hp][p0:p0 + 64, b,
                                                     moff:moff + msz],
                                            k_all[hp][p0:p0 + 64, b, :],
                                            start=True, stop=True)
                                        expP = tp7.tile([98, N], BF16,
                                                        tag='st7_exp',
                                                        name='st7_exp')
                                        ssum = sm.tile([98, 1], FP32,
                                                       tag='st7_ssum',
                                                       name='st7_ssum')
                                        nc.scalar.activation(
                                            out=expP[:], in_=s_ps[:],
                                            func=AF.Exp, scale=0.125,
                                            accum_out=ssum[:])
                                        srec = sm.tile([98, 1], FP32,
                                                       tag='st7_srec',
                                                       name='st7_srec')
                                        nc.vector.reciprocal(out=srec[:],
                                                             in_=ssum[:])
                                        nc.vector.tensor_scalar(
                                            out=expP[:], in0=expP[:],
                                            scalar1=srec[:], scalar2=None,
                                            op0=ALU.mult)
                                        for ni, (noff, nsz) in enumerate(NCK):
                                            ptp = pp7.tile([128, 98], BF16,
                                                           tag='tp128',
                                                           name='st7_ptp')
                                            nc.tensor.transpose(
                                                ptp[:nsz, :],
                                                expP[:, noff:noff + nsz],
                                                id16[:98, :98])
                                            nc.vector.tensor_copy(
                                                out=PT[ni][:nsz,
                                                           moff:moff + msz],
                                                in_=ptp[:nsz, :])
                                    for ni, (noff, nsz) in enumerate(NCK):
                                        nc.tensor.matmul(
                                            o_ps[p0:p0 + 64, :],
                                            vT_all[:nsz, b, ni,
                                                   hd * 64:(hd + 1) * 64],
                                            PT[ni][:nsz, :],
                                            start=(ni == 0), stop=(ni == 1))
                                nc.vector.tensor_copy(out=aop[:, hp, bi, :],
                                                      in_=o_ps[:])
                        ybf = tp7.tile([128, NCH, 2, N], BF16, tag='st7_ybf',
                                       name='st7_ybf')
                        for oc in range(NCH):
                            y_ps = pp7.tile([128, 392], FP32, tag='acc392',
                                            name='st7_yps')
                            for kc in range(NCH):
                                nc.tensor.matmul(
                                    y_ps[:], WoT[kc][:, oc * 128:(oc + 1) * 128],
                                    aop[:, kc, :, :],
                                    start=(kc == 0), stop=(kc == NCH - 1))
                            nc.scalar.activation(
                                out=ybf[:, oc, :, :],
                                in_=y_ps[:].rearrange('p (b n) -> p b n', b=2),
                                func=AF.Identity,
                                bias=bias_t['bo'][:, oc:oc + 1])
                        for bi, b in enumerate((bp, bp + 1)):
                            for i, (off, nsz) in enumerate(NCK):
                                xo = tb7.tile([128, C], FP32, tag='st7_xo',
                                              name='st7_xo')
                                nc.sync.dma_start(out=xo[:nsz],
                                                  in_=x_in[b, off:off + nsz, :])
                                x2 = tb7.tile([128, C], FP32, tag='st7_x2',
                                              name='st7_x2')
                                for oc in range(NCH):
                                    ypt = pp7.tile([128, 128], BF16, tag='tp128',
                                                   name='st7_ypt')
                                    nc.tensor.transpose(
                                        ypt[:nsz, :],
                                        ybf[:, oc, bi, off:off + nsz], id16)
                                    nc.vector.tensor_add(
                                        out=x2[:nsz, oc * 128:(oc + 1) * 128],
                                        in0=ypt[:nsz, :],
                                        in1=xo[:nsz, oc * 128:(oc + 1) * 128])
                                nc.sync.dma_start(
                                    out=x2_dram[b, off:off + nsz, :],
                                    in_=x2[:nsz])
                                xn2 = tb7.tile([128, C], FP32, tag='st7_xn2',
                                               name='st7_xn2')
                                ln_norm(x2, nsz, xn2[:nsz], gbt.get('ln2'), tb7)
                                for cc in range(NCH):
                                    pt = pp7.tile([128, 128], FP32, tag='tp128',
                                                  name='st7_tps')
                                    nc.tensor.transpose(
                                        pt[:, :nsz],
                                        xn2[:nsz, cc * 128:(cc + 1) * 128],
                                        id32[:nsz, :nsz])
                                    nc.vector.tensor_copy(
                                        out=xn2T[cc][:, b, off:off + nsz],
                                        in_=pt[:, :nsz])

        # ===================== ST9: MLP + residual2 ======================
        with (
            tc.tile_pool(name='p_mlp', bufs=1) as pm,
            tc.tile_pool(name='t_mlp', bufs=2) as tp9,
            tc.tile_pool(name='ps_mlp', bufs=2, space='PSUM') as pp9,
            tc.tile_pool(name='ps_mlp2', bufs=2, space='PSUM') as pp9b,
        ):
            W1T = [pm.tile([128, MLPD], BF16, tag=f'w1_{k}', name=f'w1_{k}')
                   for k in range(NCH)]
            for k in range(NCH):
                nc.sync.dma_start(out=W1T[k][:],
                                  in_=d['W1T'][k * 128:(k + 1) * 128, :])
            W2T = [pm.tile([128, C], BF16, tag=f'w2_{m}', name=f'w2_{m}')
                   for m in range(MMCH)]
            for m in range(MMCH):
                nc.sync.dma_start(out=W2T[m][:],
                                  in_=d['W2T'][m * 128:(m + 1) * 128, :])

            for bp in range(0, BL, 2):
                h1_all = tp9.tile([128, MMCH, 392], BF16, tag='h1_all',
                                  name='h1_all')
                for mm in range(MMCH):
                    h1_ps = pp9.tile([128, 392], FP32, tag='h1ps', name='h1ps')
                    for kc in range(NCH):
                        nc.tensor.matmul(
                            h1_ps[:], W1T[kc][:, mm * 128:(mm + 1) * 128],
                            xn2T[kc][:, bp:bp + 2, :],
                            start=(kc == 0), stop=(kc == NCH - 1))
                    nc.scalar.activation(out=h1_all[:, mm, :], in_=h1_ps[:],
                                         func=AF.Gelu,
                                         bias=bias_t['b1'][:, mm:mm + 1],
                                         scale=1.0)
                m2b = tp9.tile([128, NCH, 2, N], BF16, tag='st9_m2b',
                               name='st9_m2b')
                for oc in range(NCH):
                    m2_ps = pp9.tile([128, 392], FP32, tag='acc392',
                                     name='m2ps')
                    for mm in range(MMCH):
                        nc.tensor.matmul(
                            m2_ps[:], W2T[mm][:, oc * 128:(oc + 1) * 128],
                            h1_all[:, mm, :],
                            start=(mm == 0), stop=(mm == MMCH - 1))
                    nc.scalar.activation(
                        out=m2b[:, oc, :, :],
                        in_=m2_ps[:].rearrange('p (b n) -> p b n', b=2),
                        func=AF.Identity, bias=bias_t['b2'][:, oc:oc + 1])
                for bi, b in enumerate((bp, bp + 1)):
                    for i, (off, nsz) in enumerate(NCK):
                        x2r = tp9.tile([128, C], FP32, tag='st9_x2r',
                                       name='st9_x2r')
                        nc.sync.dma_start(out=x2r[:nsz],
                                          in_=x2_dram[b, off:off + nsz, :])
                        ot = tp9.tile([128, C], FP32, tag='st9_out',
                                      name='st9_out')
                        for oc in range(NCH):
                            mpt = pp9b.tile([128, 128], BF16, tag='tp128',
                                            name='st9_mpt')
                            nc.tensor.transpose(
                                mpt[:nsz, :], m2b[:, oc, bi, off:off + nsz],
                                id16)
                            nc.vector.tensor_add(
                                out=ot[:nsz, oc * 128:(oc + 1) * 128],
                                in0=mpt[:nsz, :],
                                in1=x2r[:nsz, oc * 128:(oc + 1) * 128])
                        nc.sync.dma_start(out=out_dram[b, off:off + nsz, :],
                                          in_=ot[:nsz])


def _conv_offset_gather(nc, tc, d, h, bp, qp, dwdiag, SelW, E8, id16, bias_t,
                        refy_t, refx_t, rowi_t, colj_t, wyb, wxb, oeps_t,
                        gbt, sampled, xnT_dram, tp3, sm, pp3, pp3b):
    """ST3 (conv) + ST4 (offset head, gather indices) + ST5 (gather) for a
    batch pair (bp, bp+1)."""
    ocT = {b: tp3.tile([128, 2, C], BF16, tag='st3_ocT', name='st3_ocT', bufs=2)
           for b in (bp, bp + 1)}
    for oc in range(NCH):
        for bi, b in enumerate((bp, bp + 1)):
            cv_ps = pp3.tile([128, 256], FP32, tag='acc392c', name='st3_ps')
            for t in range(9):
                ky, kx = divmod(t, 3)
                d0 = b * QPW + 16 * ky + kx
                nc.tensor.matmul(
                    cv_ps[:],
                    dwdiag[:, (t * NCH + oc) * 128:(t * NCH + oc + 1) * 128],
                    qp[oc][:, d0:d0 + 256],
                    start=(t == 0), stop=(t == 8))
            cvb = tp3.tile([128, N], BF16, tag='st3_cvb', name='st3_cvb')
            base = cv_ps[:, 0:1]
            inap = bass.AP(tensor=base.tensor, offset=base.offset,
                           ap=[base.ap[0], [16, 14], [1, 14]])
            nc.scalar.activation(out=cvb[:], in_=inap, func=AF.Identity,
                                 bias=bias_t['dwb'][:, oc:oc + 1])
            for i, (off, nsz) in enumerate(NCK):
                pt = pp3b.tile([128, 128], BF16, tag='tp128c', name='st3_tp')
                nc.tensor.transpose(pt[:nsz, :], cvb[:, off:off + nsz], id16)
                nc.vector.tensor_copy(
                    out=ocT[b][:nsz, i, oc * 128:(oc + 1) * 128], in_=pt[:nsz, :])

    idx4p = [sm.tile([128, 2, NTAP, G], BF16, tag=f'idx4_{i}', name=f'idx4_{i}')
             for i in range(2)]
    W48 = {}
    for bi, b in enumerate((bp, bp + 1)):
        W48[b] = sm.tile([G, NTAP * NPAD], BF16, tag='w48', name='w48')
        nc.vector.memset(W48[b][:], 0.0)
        for i, (off, nsz) in enumerate(NCK):
            sl = ocT[b][:nsz, i, :]
            st8 = sm.tile([128, G, 6], FP32, tag='off_st', name='off_st')
            mv8 = sm.tile([128, G, 2], FP32, tag='off_mv', name='off_mv')
            for g in range(G):
                nc.vector.bn_stats(out=st8[:nsz, g, :],
                                   in_=sl[:, g * CG:(g + 1) * CG])
                nc.vector.bn_aggr(out=mv8[:nsz, g, :], in_=st8[:nsz, g, :])
            std8 = sm.tile([128, G], FP32, tag='off_std', name='off_std')
            nc.scalar.activation(out=std8[:nsz], in_=mv8[:nsz, :, 1],
                                 func=AF.Sqrt, bias=oeps_t[:nsz], scale=1.0)
            rec8 = sm.tile([128, G], FP32, tag='off_rec', name='off_rec')
            nc.vector.reciprocal(out=rec8[:nsz], in_=std8[:nsz])
            og = tp3.tile([128, C], BF16, tag='off_og', name='off_og', bufs=2)
            ogv = og[:nsz].rearrange('p (g c) -> p g c', g=G)
            nc.vector.tensor_tensor(out=ogv,
                                    in0=sl.rearrange('p (g c) -> p g c', g=G),
                                    in1=_free_bcast(mv8[:nsz, :, 0], CG),
                                    op=ALU.subtract)
            nc.vector.tensor_tensor(out=ogv, in0=ogv,
                                    in1=_free_bcast(rec8[:nsz], CG), op=ALU.mult)
            if not h['offln_trivial']:
                gt, bt = gbt['offln']
                nc.vector.tensor_mul(out=og[:nsz], in0=og[:nsz], in1=gt[:nsz])
                nc.vector.tensor_add(out=og[:nsz], in0=og[:nsz], in1=bt[:nsz])
            nc.scalar.activation(out=og[:nsz], in_=og[:nsz], func=AF.Gelu)
            oyx = sm.tile([128, 16], FP32, tag='off_oyx', name='off_oyx')
            tpm = tp3.tile([128, C], BF16, tag='off_tpm', name='off_tpm', bufs=2)
            nc.vector.tensor_mul(out=tpm[:nsz], in0=og[:nsz], in1=wyb[:nsz])
            nc.vector.tensor_reduce(
                out=oyx[:nsz, 0:G],
                in_=tpm[:nsz].rearrange('p (g c) -> p g c', g=G),
                axis=mybir.AxisListType.X, op=ALU.add)
            nc.vector.tensor_mul(out=tpm[:nsz], in0=og[:nsz], in1=wxb[:nsz])
            nc.vector.tensor_reduce(
                out=oyx[:nsz, G:16],
                in_=tpm[:nsz].rearrange('p (g c) -> p g c', g=G),
                axis=mybir.AxisListType.X, op=ALU.add)
            th = sm.tile([128, 16], FP32, tag='off_th', name='off_th')
            nc.scalar.activation(out=th[:nsz], in_=oyx[:nsz], func=AF.Tanh)
            gy = sm.tile([128, G], FP32, tag='off_gy', name='off_gy')
            gx = sm.tile([128, G], FP32, tag='off_gx', name='off_gx')
            nc.vector.tensor_scalar(out=gy[:nsz], in0=th[:nsz, 0:G],
                                    scalar1=6.5 / 14.0, scalar2=refy_t[i][:],
                                    op0=ALU.mult, op1=ALU.add)
            nc.vector.tensor_scalar(out=gx[:nsz], in0=th[:nsz, G:16],
                                    scalar1=6.5 / 14.0, scalar2=refx_t[i][:],
                                    op0=ALU.mult, op1=ALU.add)
            fy = sm.tile([128, G], FP32, tag='off_fy', name='off_fy')
            fx = sm.tile([128, G], FP32, tag='off_fx', name='off_fx')
            y0 = sm.tile([128, G], FP32, tag='off_y0', name='off_y0')
            x0 = sm.tile([128, G], FP32, tag='off_x0', name='off_x0')
            # floor(gy) = rowi - [gy < rowi]  (exact: |offset| < 0.5 px)
            nc.vector.tensor_scalar(out=y0[:nsz], in0=gy[:nsz],
                                    scalar1=rowi_t[i][:], scalar2=None,
                                    op0=ALU.is_lt)
            nc.vector.tensor_scalar(out=y0[:nsz], in0=y0[:nsz], scalar1=-1.0,
                                    scalar2=rowi_t[i][:], op0=ALU.mult,
                                    op1=ALU.add)
            nc.vector.tensor_scalar(out=x0[:nsz], in0=gx[:nsz],
                                    scalar1=colj_t[i][:], scalar2=None,
                                    op0=ALU.is_lt)
            nc.vector.tensor_scalar(out=x0[:nsz], in0=x0[:nsz], scalar1=-1.0,
                                    scalar2=colj_t[i][:], op0=ALU.mult,
                                    op1=ALU.add)
            nc.vector.tensor_scalar_min(out=y0[:nsz], in0=y0[:nsz],
                                        scalar1=12.0)
            nc.vector.tensor_scalar_min(out=x0[:nsz], in0=x0[:nsz],
                                        scalar1=12.0)
            nc.vector.tensor_sub(out=fy[:nsz], in0=gy[:nsz], in1=y0[:nsz])
            nc.vector.tensor_sub(out=fx[:nsz], in0=gx[:nsz], in1=x0[:nsz])
            ia = sm.tile([128, G], FP32, tag='off_ia', name='off_ia')
            nc.vector.scalar_tensor_tensor(out=ia[:nsz], in0=y0[:nsz],
                                           scalar=14.0, in1=x0[:nsz],
                                           op0=ALU.mult, op1=ALU.add)
            nc.vector.tensor_copy(out=idx4p[i][:nsz, bi, 0, :], in_=ia[:nsz])
            for t, ofs in ((1, 14.0), (2, 1.0), (3, 15.0)):
                nc.vector.tensor_scalar_add(out=idx4p[i][:nsz, bi, t, :],
                                            in0=ia[:nsz], scalar1=ofs)
            fy1 = sm.tile([128, G], FP32, tag='off_fy1', name='off_fy1')
            fx1 = sm.tile([128, G], FP32, tag='off_fx1', name='off_fx1')
            nc.vector.tensor_scalar(out=fy1[:nsz], in0=fy[:nsz], scalar1=-1.0,
                                    scalar2=1.0, op0=ALU.mult, op1=ALU.add)
            nc.vector.tensor_scalar(out=fx1[:nsz], in0=fx[:nsz], scalar1=-1.0,
                                    scalar2=1.0, op0=ALU.mult, op1=ALU.add)
            for t, (aa, bb) in enumerate(((fx1, fy1), (fx1, fy),
                                          (fx, fy1), (fx, fy))):
                wt = sm.tile([128, G], BF16, tag='off_wt', name='off_wt')
                nc.vector.tensor_mul(out=wt[:nsz], in0=aa[:nsz], in1=bb[:nsz])
                ptw = pp3b.tile([G, 128], BF16, tag='tp128c', name='off_ptw')
                nc.tensor.transpose(ptw[:, :nsz], wt[:nsz], id16[:nsz, :nsz])
                nc.vector.tensor_copy(
                    out=W48[b][:, t * NPAD + off:t * NPAD + off + nsz],
                    in_=ptw[:, :nsz])

    wrapP = pp3.tile([128, 13 * 64], FP32, tag='wrapps', name='st4_wrap', bufs=1)
    for f in range(13):
        for i in range(2):
            nsz = NCK[i][1]
            nc.tensor.matmul(wrapP[:, f * 64:(f + 1) * 64],
                             SelW[i][:nsz, f * 128:(f + 1) * 128],
                             idx4p[i][:nsz].rearrange('p b t g -> p (b t g)'),
                             start=(i == 0), stop=(i == 1))
    # reorder psum (f,b,t,g) -> sbuf (g,b,t,f) while casting to int16
    wrapS = sm.tile([128, G, 2, NTAP, 13], I16, tag='wrapS', name='wrapS')
    for bb in range(2):
        base_in = wrapP[:, 0:1]
        inap = bass.AP(tensor=base_in.tensor, offset=base_in.offset + bb * 32,
                       ap=[base_in.ap[0], [64, 13], [8, NTAP], [1, G]])
        base_out = wrapS[:, 0, bb, 0, 0:1]
        outap = bass.AP(tensor=base_out.tensor, offset=base_out.offset,
                        ap=[base_out.ap[0], [1, 13], [13, NTAP], [104, G]])
        nc.vector.tensor_copy(out=outap, in_=inap)
    idxt = [sm.tile([128, 2, NTAP, 13], I16, tag=f'idxt{j}', name=f'idxt{j}')
            for j in range(NCH)]
    for j in range(NCH):
        bands_g = [(8 * j + band) // 6 for band in range(8)]
        runs = []
        r0 = 0
        for band in range(1, 9):
            if band == 8 or bands_g[band] != bands_g[r0]:
                runs.append((r0, band - 1, bands_g[r0]))
                r0 = band
        for (b0, b1, g) in runs:
            p0, pn = 16 * b0, 16 * (b1 - b0 + 1)
            nc.sync.dma_start(out=idxt[j][p0:p0 + pn],
                              in_=wrapS[p0:p0 + pn, g])

    for bi, b in enumerate((bp, bp + 1)):
        img = tp3.tile([128, NCH, N], FP32, tag='st5_img', name='st5_img', bufs=2)
        nc.sync.dma_start(out=img[:], in_=xnT_dram[b])
        for j in range(NCH):
            gth = tp3.tile([128, NTAP * NPAD], FP32, tag='st5_g', name='st5_g', bufs=2)
            nc.gpsimd.ap_gather(
                out_ap=gth[:],
                in_ap=img[:, j, :].rearrange('p (n one) -> p n one', one=1),
                idxs_ap=idxt[j][:, bi].rearrange('p t f -> p (t f)'),
                channels=128, num_elems=N, d=1, num_idxs=NTAP * NPAD)
            wb0 = pp3.tile([128, 2 * NPAD], FP32, tag='wbc', name='st5_w0')
            wb1 = pp3.tile([128, 2 * NPAD], FP32, tag='wbc', name='st5_w1')
            nc.tensor.matmul(wb0[:], E8[:, j * 128:(j + 1) * 128],
                             W48[b][:, 0:2 * NPAD], start=True, stop=True)
            nc.tensor.matmul(wb1[:], E8[:, j * 128:(j + 1) * 128],
                             W48[b][:, 2 * NPAD:4 * NPAD], start=True, stop=True)
            gw = tp3.tile([128, NTAP * NPAD], BF16, tag='st5_gw', name='st5_gw')
            nc.vector.tensor_tensor(out=gw[:, 0:2 * NPAD], in0=gth[:, 0:2 * NPAD],
                                    in1=wb0[:], op=ALU.mult)
            nc.vector.tensor_tensor(out=gw[:, 2 * NPAD:], in0=gth[:, 2 * NPAD:],
                                    in1=wb1[:], op=ALU.mult)
            s01 = tp3.tile([128, N], BF16, tag='st5_s01', name='st5_s01')
            s23 = tp3.tile([128, N], BF16, tag='st5_s23', name='st5_s23')
            nc.vector.tensor_add(out=s01[:], in0=gw[:, 0:N],
                                 in1=gw[:, NPAD:NPAD + N])
            nc.vector.tensor_add(out=s23[:], in0=gw[:, 2 * NPAD:2 * NPAD + N],
                                 in1=gw[:, 3 * NPAD:3 * NPAD + N])
            nc.vector.tensor_add(out=sampled[j][:, b, :], in0=s01[:], in1=s23[:])


def build_nc(h):
    from concourse import bacc
    nc = bacc.Bacc(None, target_bir_lowering=False, debug=False)
    d = {}

    def din(name, shape, dt):
        d[name] = nc.declare_dram_parameter(name, list(shape), dt, isOutput=False)

    din('x_shard', (BL, N, C), FP32)
    out_dram = nc.declare_dram_parameter('out', [BL, N, C], FP32, isOutput=True)
    x2_dram = nc.dram_tensor('x2_scratch', [BL, N, C], FP32)
    xnT_dram = nc.dram_tensor('xnT_scratch', [BL, 128, NCH, N], FP32)

    din('WqT', (C, C), BF16); din('WkT', (C, C), BF16)
    din('WvT', (C, C), BF16); din('WoT', (C, C), BF16)
    din('W1T', (C, MLPD), BF16); din('W2T', (MLPD, C), BF16)
    din('bq', (128, NCH), FP32); din('bk', (128, NCH), FP32)
    din('bo', (128, NCH), FP32); din('b1', (128, MMCH), FP32)
    din('b2', (128, NCH), FP32)
    din('dwdiag', (9, NCH, 128, 128), BF16); din('dwb', (128, NCH), FP32)
    din('E8', (G, C), BF16); din('SelW', (2, 128, 13 * 128), BF16)
    din('refy', (N,), FP32); din('refx', (N,), FP32)
    din('rowi', (N,), FP32); din('colj', (N,), FP32)
    din('wyv', (C,), BF16); din('wxv', (C,), BF16)
    din('id32', (128, 128), FP32); din('id16', (128, 128), BF16)
    if not h['ln1_trivial']:
        din('ln1_g', (C,), FP32); din('ln1_b', (C,), FP32)
    if not h['ln2_trivial']:
        din('ln2_g', (C,), FP32); din('ln2_b', (C,), FP32)
    if not h['offln_trivial']:
        din('offln_g', (C,), FP32); din('offln_b', (C,), FP32)
    if not h['bv_trivial']:
        din('bv', (128, C), FP32)

    with tile.TileContext(nc) as tc:
        emit(nc, tc, d, out_dram, x2_dram, xnT_dram, h)
    nc.compile()
    return nc


_DECLARED = {'WqT', 'WkT', 'WvT', 'WoT', 'W1T', 'W2T', 'bq', 'bk', 'bo',
             'b1', 'b2', 'dwdiag', 'dwb', 'E8', 'SelW', 'refy', 'refx',
             'wyv', 'wxv', 'id32', 'id16', 'rowi', 'colj'}

_CACHE = {}


def kernel(**inputs):
    h = build_host_consts(inputs)
    if 'nc' not in _CACHE:
        _CACHE['nc'] = build_nc(h)
    nc = _CACHE['nc']

    declared = set(_DECLARED)
    for nm in ('ln1', 'ln2', 'offln'):
        if not h[nm + '_trivial']:
            declared |= {nm + '_g', nm + '_b'}
    if not h['bv_trivial']:
        declared.add('bv')
    shared = {k: v for k, v in h.items()
              if k in declared and isinstance(v, np.ndarray)}

    x = _f32(inputs['x'])
    in_maps = []
    for c in range(NCORES):
        m = dict(shared)
        m['x_shard'] = np.ascontiguousarray(x[c * BL:(c + 1) * BL])
        in_maps.append(m)
    res = run_bass_kernel_spmd(nc, in_maps, list(range(NCORES)))
    outs = [res.results[c]['out'] for c in range(NCORES)]
    return np.concatenate(outs, axis=0).astype(np.float32)



# revision 26
# speedup vs baseline: 1.6986x; 1.6986x over previous
"""Trainium2 Bass kernel for a DAT-style transformer block (sparse_attention).

kernel(**inputs) takes FULL unsharded inputs (B=64), shards the batch across
8 NeuronCores (8 per core, pure data parallel — no collectives), runs one SPMD
Bass/Tile program, returns the FULL [64, 196, 768] float32 output.

Per-core pipeline (8 local batches):
  ST1 LN1 + PE transposes -> xnTb (bf16) + xnpad (bf16, zero-padded 16x16 grid)
  ST2 q = Wq@xnT + bq  -> qp, padded [16,16] spatial layout (bf16)
  ST3 depthwise 3x3 conv via 9 accumulating diag-matmuls; transpose -> ocT
  ST4 offset head: group-LN -> GELU -> proj -> tanh -> pixel coords ->
      3x3 separable hat weights per (group, pixel) (exact bilinear: |offset|
      < 1 px keeps all 4 taps inside the 3x3 stencil; pad border = zeros)
  ST5 expand weights group->channel (PE matmul vs E8), 9 shifted multiplies
      on xnpad + reduce (DVE) -> sampled
  ST6 k = Wk@sampled + bk; vT = sampled^T @ Wv^T
  ST7 attention per head (S -> exp(+rowsum) -> norm -> P^T -> @v), o-proj,
      residual 1, LN2 (x2 spilled to DRAM), transposes -> xn2T
  ST9 MLP (24x fused h1 -> GELU -> accumulate m2), bias, transpose, residual 2
All matmuls bf16 with fp32 PSUM accumulation; stats/softmax/residuals fp32.
"""

import numpy as np
import ml_dtypes

import concourse.bass as bass
import concourse.mybir as mybir
import concourse.tile as tile
from concourse import library_config
from concourse.bass_utils import run_bass_kernel_spmd

FP32 = mybir.dt.float32
BF16 = mybir.dt.bfloat16
I16 = mybir.dt.int16
AF = mybir.ActivationFunctionType
ALU = mybir.AluOpType

B = 64
NCORES = 8
BL = 8
N = 196
C = 768
NCH = 6
HEADS = 12
HD = 64
G = 8
CG = 96
MLPD = 3072
MMCH = 24
HH = 14
NCK = [(0, 128), (128, 68)]
MCK = [(0, 98), (98, 98)]
EPS = 1e-6
OFF_EPS = 1e-5
NPAD = 208
NTAP = 4
QPW = 290
DEBUG = False


def _f32(x):
    return np.ascontiguousarray(np.asarray(x), dtype=np.float32)


def _bf16(x):
    return np.ascontiguousarray(
        np.asarray(x, dtype=np.float32).astype(ml_dtypes.bfloat16))


def build_host_consts(inp):
    h = {}
    h['WqT'] = _bf16(np.asarray(inp['Wq'], np.float32).T)
    h['WkT'] = _bf16(np.asarray(inp['Wk'], np.float32).T)
    h['WvT'] = _bf16(np.asarray(inp['Wv'], np.float32).T)
    h['WoT'] = _bf16(np.asarray(inp['Wo'], np.float32).T)
    h['W1T'] = _bf16(np.asarray(inp['W1'], np.float32).T)
    h['W2T'] = _bf16(np.asarray(inp['W2'], np.float32).T)

    h['bq'] = _f32(np.asarray(inp['bq']).reshape(NCH, 128).T)
    h['bk'] = _f32(np.asarray(inp['bk']).reshape(NCH, 128).T)
    h['bo'] = _f32(np.asarray(inp['bo']).reshape(NCH, 128).T)
    h['b1'] = _f32(np.asarray(inp['b1']).reshape(MMCH, 128).T)
    h['b2'] = _f32(np.asarray(inp['b2']).reshape(NCH, 128).T)

    dw = np.asarray(inp['off_dw_w'], np.float32).reshape(CG, 9)
    dwg = np.tile(dw, (G, 1))
    diag = np.zeros((9, NCH, 128, 128), np.float32)
    for t in range(9):
        for cc in range(NCH):
            np.fill_diagonal(diag[t, cc], dwg[cc * 128:(cc + 1) * 128, t])
    h['dwdiag'] = _bf16(diag)
    h['dwb'] = _f32(np.tile(np.asarray(inp['off_dw_b'], np.float32), G)
                    .reshape(NCH, 128).T)

    e8 = np.zeros((G, C), np.float32)
    for c in range(C):
        e8[c // CG, c] = 1.0
    h['E8'] = _bf16(e8)

    ii = np.arange(HH, dtype=np.float32)
    h['refy'] = _f32(np.repeat((ii + 0.5) * 13.0 / 14.0, HH))
    h['refx'] = _f32(np.tile((ii + 0.5) * 13.0 / 14.0, HH))
    # rc3[n, d*16+g] = row(n)+(d-1) for g<8 else col(n)+(d-1): hat centers
    rowi = np.repeat(ii, HH)
    colj = np.tile(ii, HH)
    rc3 = np.zeros((N, 48), np.float32)
    for dd in range(3):
        rc3[:, dd * 16:dd * 16 + 8] = (rowi + dd - 1)[:, None]
        rc3[:, dd * 16 + 8:dd * 16 + 16] = (colj + dd - 1)[:, None]
    h['rc3'] = _f32(rc3)

    pw = np.asarray(inp['off_proj_w'], np.float32)
    h['wyv'] = _bf16(np.tile(pw[0], G))
    h['wxv'] = _bf16(np.tile(pw[1], G))

    h['id32'] = _f32(np.eye(128, dtype=np.float32))
    h['id16'] = _bf16(np.eye(128, dtype=np.float32))

    for nm, gk, bk_ in (('ln1', 'ln1_g', 'ln1_b'), ('ln2', 'ln2_g', 'ln2_b')):
        g = np.asarray(inp[gk], np.float32)
        bb = np.asarray(inp[bk_], np.float32)
        h[nm + '_trivial'] = bool(np.all(g == 1.0) and np.all(bb == 0.0))
        h[nm + '_g'] = _f32(g)
        h[nm + '_b'] = _f32(bb)
    og = np.tile(np.asarray(inp['off_ln_g'], np.float32), G)
    ob = np.tile(np.asarray(inp['off_ln_b'], np.float32), G)
    h['offln_trivial'] = bool(np.all(og == 1.0) and np.all(ob == 0.0))
    h['offln_g'] = _f32(og)
    h['offln_b'] = _f32(ob)
    bv = np.asarray(inp['bv'], np.float32)
    h['bv_trivial'] = bool(np.all(bv == 0.0))
    h['bv'] = _f32(np.tile(bv.reshape(1, C), (128, 1)))
    return h


def _free_bcast(t_ap, inner):
    """View [P, F] AP as [P, F, inner] with a stride-0 inner dim."""
    return bass.AP(tensor=t_ap.tensor, offset=t_ap.offset,
                   ap=list(t_ap.ap) + [[0, inner]])


def _dram_bcast(src_ap, rows):
    return bass.AP(tensor=src_ap.tensor, offset=src_ap.offset,
                   ap=[[0, rows]] + list(src_ap.ap))


def emit(nc, tc, d, out_dram, x2_dram, h):
    x_in = d['x_shard']

    with (
        tc.tile_pool(name='cw', bufs=1) as cw,
        tc.tile_pool(name='p_xn2T', bufs=1) as p_xn2T,
        tc.tile_pool(name='p_sm', bufs=4) as sm,
    ):
        # ---- always-resident constants --------------------------------
        WoT = [cw.tile([128, C], BF16, tag=f'wo{k}', name=f'wo{k}')
               for k in range(NCH)]
        for k in range(NCH):
            nc.sync.dma_start(out=WoT[k][:], in_=d['WoT'][k * 128:(k + 1) * 128, :])
        E8 = cw.tile([G, C], BF16, tag='e8', name='e8')
        nc.sync.dma_start(out=E8[:], in_=d['E8'][:])
        id32 = cw.tile([128, 128], FP32, tag='id32', name='id32')
        id16 = cw.tile([128, 128], BF16, tag='id16', name='id16')
        nc.sync.dma_start(out=id32[:], in_=d['id32'][:])
        nc.sync.dma_start(out=id16[:], in_=d['id16'][:])
        bias_t = {}
        for nm, cols in (('bq', NCH), ('bk', NCH), ('bo', NCH), ('b1', MMCH),
                         ('b2', NCH), ('dwb', NCH)):
            bias_t[nm] = cw.tile([128, cols], FP32, tag='bias_' + nm,
                                 name='bias_' + nm)
            nc.sync.dma_start(out=bias_t[nm][:], in_=d[nm][:])
        refy_t, refx_t, rc3_t = [], [], []
        for i, (off, nsz) in enumerate(NCK):
            for nm, lst in (('refy', refy_t), ('refx', refx_t)):
                tt = cw.tile([nsz, 1], FP32, tag=f'{nm}{i}', name=f'{nm}{i}')
                nc.sync.dma_start(
                    out=tt[:],
                    in_=d[nm][off:off + nsz].rearrange('(n one) -> n one',
                                                       one=1))
                lst.append(tt)
            rt = cw.tile([nsz, 3, 16], FP32, tag=f'rc3{i}', name=f'rc3{i}')
            nc.sync.dma_start(
                out=rt[:],
                in_=d['rc3'][off:off + nsz].rearrange('n (d g) -> n d g', d=3))
            rc3_t.append(rt)
        wyb = cw.tile([128, C], BF16, tag='wyb', name='wyb')
        wxb = cw.tile([128, C], BF16, tag='wxb', name='wxb')
        nc.sync.dma_start(out=wyb[:], in_=_dram_bcast(d['wyv'][:], 128))
        nc.sync.dma_start(out=wxb[:], in_=_dram_bcast(d['wxv'][:], 128))
        eps_t = cw.tile([128, 1], FP32, tag='eps', name='eps')
        nc.vector.memset(eps_t[:], EPS)
        oeps_t = cw.tile([128, 1], FP32, tag='oeps', name='oeps')
        nc.vector.memset(oeps_t[:], OFF_EPS)
        gbt = {}
        for nm in ('ln1', 'ln2', 'offln'):
            if not h[nm + '_trivial']:
                g_ = cw.tile([128, C], FP32, tag=nm + 'g', name=nm + 'g')
                b_ = cw.tile([128, C], FP32, tag=nm + 'b', name=nm + 'b')
                nc.sync.dma_start(out=g_[:], in_=_dram_bcast(d[nm + '_g'][:], 128))
                nc.sync.dma_start(out=b_[:], in_=_dram_bcast(d[nm + '_b'][:], 128))
                gbt[nm] = (g_, b_)
        bv_t = None
        if not h['bv_trivial']:
            bv_t = cw.tile([128, C], FP32, tag='bvt', name='bvt')
            nc.sync.dma_start(out=bv_t[:], in_=d['bv'][:])

        def ln_norm(xf, nsz, out_ap, gbk, tmp_pool):
            st = sm.tile([128, 3, 6], FP32, tag='ln_st', name='ln_st')
            for s in range(3):
                nc.vector.bn_stats(out=st[:nsz, s, :],
                                   in_=xf[:nsz, s * 256:(s + 1) * 256])
            mv = sm.tile([128, 2], FP32, tag='ln_mv', name='ln_mv')
            nc.vector.bn_aggr(out=mv[:nsz], in_=st[:nsz])
            std = sm.tile([128, 1], FP32, tag='ln_std', name='ln_std')
            nc.scalar.activation(out=std[:nsz], in_=mv[:nsz, 1:2], func=AF.Sqrt,
                                 bias=eps_t[:nsz], scale=1.0)
            rstd = sm.tile([128, 1], FP32, tag='ln_rstd', name='ln_rstd')
            nc.vector.reciprocal(out=rstd[:nsz], in_=std[:nsz])
            nmr = sm.tile([128, 1], FP32, tag='ln_nmr', name='ln_nmr')
            nc.vector.tensor_scalar(out=nmr[:nsz], in0=mv[:nsz, 0:1],
                                    scalar1=rstd[:nsz], scalar2=-1.0,
                                    op0=ALU.mult, op1=ALU.mult)
            if gbk is None:
                nc.scalar.activation(out=out_ap, in_=xf[:nsz], func=AF.Identity,
                                     bias=nmr[:nsz], scale=rstd[:nsz])
            else:
                gt, bt = gbk
                tmp = tmp_pool.tile([128, C], FP32, tag='ln_tmp', name='ln_tmp')
                nc.scalar.activation(out=tmp[:nsz], in_=xf[:nsz], func=AF.Identity,
                                     bias=nmr[:nsz], scale=rstd[:nsz])
                nc.vector.tensor_mul(out=tmp[:nsz], in0=tmp[:nsz], in1=gt[:nsz])
                nc.vector.tensor_add(out=out_ap, in0=tmp[:nsz], in1=bt[:nsz])

        xn2T = [p_xn2T.tile([128, BL, N], BF16, tag=f'x2T{k}', name=f'x2T{k}')
                for k in range(NCH)]

        with tc.tile_pool(name='p_qp', bufs=1) as p_qp:
            q_pl = [p_qp.tile([128, BL, N], BF16, tag=f'qpl{k}', name=f'qpl{k}')
                    for k in range(NCH)]
            with tc.tile_pool(name='p_smp', bufs=1) as p_smp:
                sampled = [p_smp.tile([128, BL, N], BF16, tag=f'smp{k}',
                                      name=f'smp{k}') for k in range(NCH)]
                with tc.tile_pool(name='p_qpad', bufs=1) as p_qpad:
                    qp = [p_qpad.tile([128, BL * QPW], BF16, tag=f'qp{k}',
                                      name=f'qp{k}') for k in range(NCH)]
                    xnpad = [p_qpad.tile([128, BL * QPW], BF16, tag=f'xnp{k}',
                                         name=f'xnp{k}') for k in range(NCH)]
                    for k in range(NCH):
                        nc.vector.memset(qp[k][:], 0.0)
                        nc.vector.memset(xnpad[k][:], 0.0)
                    with (
                        tc.tile_pool(name='p_st12', bufs=1) as p12,
                        tc.tile_pool(name='t_st12', bufs=3) as tp1,
                        tc.tile_pool(name='tb_st12', bufs=2) as tb1,
                        tc.tile_pool(name='ps_st12', bufs=3, space='PSUM') as pp1,
                        tc.tile_pool(name='ps_st12b', bufs=2, space='PSUM') as pp1b,
                    ):
                        WqT = [p12.tile([128, C], BF16, tag=f'wq{k}', name=f'wq{k}')
                               for k in range(NCH)]
                        for k in range(NCH):
                            nc.sync.dma_start(out=WqT[k][:],
                                              in_=d['WqT'][k * 128:(k + 1) * 128, :])
                        xnTb = [p12.tile([128, BL, N], BF16, tag=f'xnTb{k}',
                                         name=f'xnTb{k}') for k in range(NCH)]

                        for b in range(BL):
                            for i, (off, nsz) in enumerate(NCK):
                                xf = tb1.tile([128, C], FP32, tag='st1_x', name='st1_x')
                                nc.sync.dma_start(out=xf[:nsz],
                                                  in_=x_in[b, off:off + nsz, :])
                                xn = tb1.tile([128, C], FP32, tag='st1_xn', name='st1_xn')
                                ln_norm(xf, nsz, xn[:nsz], gbt.get('ln1'), tb1)
                                for cc in range(NCH):
                                    pt = pp1.tile([128, 128], FP32, tag='tp128',
                                                  name='st1_ps')
                                    nc.tensor.transpose(
                                        pt[:, :nsz], xn[:nsz, cc * 128:(cc + 1) * 128],
                                        id32[:nsz, :nsz])
                                    nc.vector.tensor_copy(
                                        out=xnTb[cc][:, b, off:off + nsz],
                                        in_=pt[:, :nsz])
                        # pad xnTb into the zeroed 16x16 grid (all 8 batches,
                        # one scalar copy per channel chunk)
                        for cc in range(NCH):
                            sb = xnTb[cc][:, 0, 0:1]
                            src = bass.AP(tensor=sb.tensor, offset=sb.offset,
                                          ap=[sb.ap[0], [N, BL], [14, 14], [1, 14]])
                            db = xnpad[cc][:, 17:18]
                            dst = bass.AP(tensor=db.tensor, offset=db.offset,
                                          ap=[db.ap[0], [QPW, BL], [16, 14], [1, 14]])
                            nc.scalar.activation(out=dst, in_=src, func=AF.Identity)

                        for bp in range(0, BL, 2):
                            for oc in range(NCH):
                                q_ps = pp1b.tile([128, 392], FP32, tag='acc392',
                                                 name='st2_ps')
                                for kc in range(NCH):
                                    nc.tensor.matmul(
                                        q_ps[:], WqT[kc][:, oc * 128:(oc + 1) * 128],
                                        xnTb[kc][:, bp:bp + 2, :],
                                        start=(kc == 0), stop=(kc == NCH - 1))
                                for bi, b in enumerate((bp, bp + 1)):
                                    base = qp[oc][:, b * QPW + 17:b * QPW + 18]
                                    outap = bass.AP(tensor=base.tensor,
                                                    offset=base.offset,
                                                    ap=[base.ap[0], [16, 14], [1, 14]])
                                    nc.scalar.activation(
                                        out=outap,
                                        in_=q_ps[:, bi * N:(bi + 1) * N],
                                        func=AF.Identity,
                                        bias=bias_t['bq'][:, oc:oc + 1])
                                nc.vector.tensor_copy(
                                    out=q_pl[oc][:, bp:bp + 2, :],
                                    in_=q_ps[:].rearrange('p (b n) -> p b n', b=2))

                    # ============ ST3..ST5 (pair loop), then ST6, ST7 ===========
                    with (
                        tc.tile_pool(name='p_cs', bufs=1) as pcs,
                        tc.tile_pool(name='t_cs', bufs=3) as tp3,
                        tc.tile_pool(name='ps_cs', bufs=2, space='PSUM') as pp3,
                        tc.tile_pool(name='ps_cs2', bufs=2, space='PSUM') as pp3b,
                    ):
                        dwdiag = pcs.tile([128, 9 * NCH * 128], BF16, tag='dwdiag',
                                          name='dwdiag')
                        nc.sync.dma_start(
                            out=dwdiag[:].rearrange('p (t c m) -> p t c m',
                                                    t=9, c=NCH),
                            in_=d['dwdiag'][:].rearrange('t c p m -> p t c m'))

                        for bp in range(0, BL, 2):
                            _conv_offset_sample(
                                nc, tc, d, h, bp, qp, xnpad, dwdiag, E8, id32,
                                id16, bias_t, refy_t, refx_t, rc3_t,
                                wyb, wxb, oeps_t, gbt,
                                sampled, tp3, sm, pp3, pp3b)
                        if DEBUG:
                            for j in range(NCH):
                                nc.sync.dma_start(out=d['dbg_xnp'][j],
                                                  in_=xnpad[j][:])
                                nc.sync.dma_start(out=d['dbg_smp'][j],
                                                  in_=sampled[j][:])

                with (
                    tc.tile_pool(name='p_kv', bufs=1) as pkv,
                    tc.tile_pool(name='t_67', bufs=3) as tp7,
                    tc.tile_pool(name='tb_67', bufs=2) as tb7,
                    tc.tile_pool(name='ps_67', bufs=2, space='PSUM') as pp7,
                    tc.tile_pool(name='ps_67b', bufs=2, space='PSUM') as pp7b,
                ):
                    WkT = [pkv.tile([128, C], BF16, tag=f'wk{k}', name=f'wk{k}')
                           for k in range(NCH)]
                    WvT = [pkv.tile([128, C], BF16, tag=f'wv{k}', name=f'wv{k}')
                           for k in range(NCH)]
                    for k in range(NCH):
                        nc.sync.dma_start(out=WkT[k][:],
                                          in_=d['WkT'][k * 128:(k + 1) * 128, :])
                        nc.sync.dma_start(out=WvT[k][:],
                                          in_=d['WvT'][k * 128:(k + 1) * 128, :])
                    k_all = [pkv.tile([128, BL, N], BF16, tag=f'kk{k}',
                                      name=f'kk{k}') for k in range(NCH)]
                    vT_all = pkv.tile([128, BL, 2, C], BF16, tag='vT', name='vT')

                    # ---------------- ST6 --------------------------------
                    for bp in range(0, BL, 2):
                        for oc in range(NCH):
                            k_ps = pp7.tile([128, 392], FP32, tag='acc392',
                                            name='st6_kps')
                            for kc in range(NCH):
                                nc.tensor.matmul(
                                    k_ps[:], WkT[kc][:, oc * 128:(oc + 1) * 128],
                                    sampled[kc][:, bp:bp + 2, :],
                                    start=(kc == 0), stop=(kc == NCH - 1))
                            nc.scalar.activation(
                                out=k_all[oc][:, bp:bp + 2, :],
                                in_=k_ps[:].rearrange('p (b n) -> p b n', b=2),
                                func=AF.Identity,
                                bias=bias_t['bk'][:, oc:oc + 1])
                    for b in range(BL):
                        for i, (off, nsz) in enumerate(NCK):
                            for half in range(2):
                                v_ps = pp7.tile([128, 384], FP32, tag='acc392',
                                                name='st6_vps')
                                for kc in range(NCH):
                                    nc.tensor.matmul(
                                        v_ps[:nsz],
                                        sampled[kc][:, b, off:off + nsz],
                                        WvT[kc][:, half * 384:(half + 1) * 384],
                                        start=(kc == 0), stop=(kc == NCH - 1))
                                dst = vT_all[:nsz, b, i,
                                             half * 384:(half + 1) * 384]
                                if bv_t is None:
                                    nc.vector.tensor_copy(out=dst, in_=v_ps[:nsz])
                                else:
                                    nc.vector.tensor_add(
                                        out=dst, in0=v_ps[:nsz],
                                        in1=bv_t[:nsz,
                                                 half * 384:(half + 1) * 384])

                    # ---------------- ST7 --------------------------------
                    for bp in range(0, BL, 2):
                        aop = tp7.tile([128, NCH, 2, N], BF16, tag='st7_ao',
                                       name='st7_ao')
                        for bi, b in enumerate((bp, bp + 1)):
                            for hp in range(NCH):
                                o_ps = pp7b.tile([128, N], FP32, tag='st7_ops',
                                                 name='st7_ops')
                                for hh in range(2):
                                    hd = hp * 2 + hh
                                    p0 = (hd % 2) * 64
                                    PT = [tp7.tile([128, N], BF16, tag='st7_pt',
                                                   name='st7_pt')
                                          for _ in range(2)]
                                    for mi, (moff, msz) in enumerate(MCK):
                                        s_ps = pp7b.tile([98, N], FP32,
                                                         tag='st7_sps',
                                                         name='st7_sps')
                                        nc.tensor.matmul(
                                            s_ps[:],
                                            q_pl[hp][p0:p0 + 64, b,
                                                     moff:moff + msz],
                                            k_all[hp][p0:p0 + 64, b, :],
                                            start=True, stop=True)
                                        expP = tp7.tile([98, N], BF16,
                                                        tag='st7_exp',
                                                        name='st7_exp')
                                        ssum = sm.tile([98, 1], FP32,
                                                       tag='st7_ssum',
                                                       name='st7_ssum')
                                        nc.scalar.activation(
                                            out=expP[:], in_=s_ps[:],
                                            func=AF.Exp, scale=0.125,
                                            accum_out=ssum[:])
                                        srec = sm.tile([98, 1], FP32,
                                                       tag='st7_srec',
                                                       name='st7_srec')
                                        nc.vector.reciprocal(out=srec[:],
                                                             in_=ssum[:])
                                        nc.vector.tensor_scalar(
                                            out=expP[:], in0=expP[:],
                                            scalar1=srec[:], scalar2=None,
                                            op0=ALU.mult)
                                        for ni, (noff, nsz) in enumerate(NCK):
                                            ptp = pp7.tile([128, 98], BF16,
                                                           tag='tp128',
                                                           name='st7_ptp')
                                            nc.tensor.transpose(
                                                ptp[:nsz, :],
                                                expP[:, noff:noff + nsz],
                                                id16[:98, :98])
                                            nc.vector.tensor_copy(
                                                out=PT[ni][:nsz,
                                                           moff:moff + msz],
                                                in_=ptp[:nsz, :])
                                    for ni, (noff, nsz) in enumerate(NCK):
                                        nc.tensor.matmul(
                                            o_ps[p0:p0 + 64, :],
                                            vT_all[:nsz, b, ni,
                                                   hd * 64:(hd + 1) * 64],
                                            PT[ni][:nsz, :],
                                            start=(ni == 0), stop=(ni == 1))
                                nc.vector.tensor_copy(out=aop[:, hp, bi, :],
                                                      in_=o_ps[:])
                        ybf = tp7.tile([128, NCH, 2, N], BF16, tag='st7_ybf',
                                       name='st7_ybf')
                        for oc in range(NCH):
                            y_ps = pp7.tile([128, 392], FP32, tag='acc392',
                                            name='st7_yps')
                            for kc in range(NCH):
                                nc.tensor.matmul(
                                    y_ps[:], WoT[kc][:, oc * 128:(oc + 1) * 128],
                                    aop[:, kc, :, :],
                                    start=(kc == 0), stop=(kc == NCH - 1))
                            nc.scalar.activation(
                                out=ybf[:, oc, :, :],
                                in_=y_ps[:].rearrange('p (b n) -> p b n', b=2),
                                func=AF.Identity,
                                bias=bias_t['bo'][:, oc:oc + 1])
                        for bi, b in enumerate((bp, bp + 1)):
                            for i, (off, nsz) in enumerate(NCK):
                                xo = tb7.tile([128, C], FP32, tag='st7_xo',
                                              name='st7_xo')
                                nc.sync.dma_start(out=xo[:nsz],
                                                  in_=x_in[b, off:off + nsz, :])
                                x2 = tb7.tile([128, C], FP32, tag='st7_x2',
                                              name='st7_x2')
                                for oc in range(NCH):
                                    ypt = pp7.tile([128, 128], BF16, tag='tp128',
                                                   name='st7_ypt')
                                    nc.tensor.transpose(
                                        ypt[:nsz, :],
                                        ybf[:, oc, bi, off:off + nsz], id16)
                                    nc.vector.tensor_add(
                                        out=x2[:nsz, oc * 128:(oc + 1) * 128],
                                        in0=ypt[:nsz, :],
                                        in1=xo[:nsz, oc * 128:(oc + 1) * 128])
                                nc.sync.dma_start(
                                    out=x2_dram[b, off:off + nsz, :],
                                    in_=x2[:nsz])
                                xn2 = tb7.tile([128, C], FP32, tag='st7_xn2',
                                               name='st7_xn2')
                                ln_norm(x2, nsz, xn2[:nsz], gbt.get('ln2'), tb7)
                                for cc in range(NCH):
                                    pt = pp7.tile([128, 128], FP32, tag='tp128',
                                                  name='st7_tps')
                                    nc.tensor.transpose(
                                        pt[:, :nsz],
                                        xn2[:nsz, cc * 128:(cc + 1) * 128],
                                        id32[:nsz, :nsz])
                                    nc.vector.tensor_copy(
                                        out=xn2T[cc][:, b, off:off + nsz],
                                        in_=pt[:, :nsz])

        # ===================== ST9: MLP + residual2 ======================
        with (
            tc.tile_pool(name='p_mlp', bufs=1) as pm,
            tc.tile_pool(name='t_mlp', bufs=2) as tp9,
            tc.tile_pool(name='ps_mlp', bufs=2, space='PSUM') as pp9,
            tc.tile_pool(name='ps_mlp2', bufs=2, space='PSUM') as pp9b,
        ):
            W1T = [pm.tile([128, MLPD], BF16, tag=f'w1_{k}', name=f'w1_{k}')
                   for k in range(NCH)]
            for k in range(NCH):
                nc.sync.dma_start(out=W1T[k][:],
                                  in_=d['W1T'][k * 128:(k + 1) * 128, :])
            W2T = [pm.tile([128, C], BF16, tag=f'w2_{m}', name=f'w2_{m}')
                   for m in range(MMCH)]
            for m in range(MMCH):
                nc.sync.dma_start(out=W2T[m][:],
                                  in_=d['W2T'][m * 128:(m + 1) * 128, :])

            for bp in range(0, BL, 2):
                h1_all = tp9.tile([128, MMCH, 392], BF16, tag='h1_all',
                                  name='h1_all')
                for mm in range(MMCH):
                    h1_ps = pp9.tile([128, 392], FP32, tag='h1ps', name='h1ps')
                    for kc in range(NCH):
                        nc.tensor.matmul(
                            h1_ps[:], W1T[kc][:, mm * 128:(mm + 1) * 128],
                            xn2T[kc][:, bp:bp + 2, :],
                            start=(kc == 0), stop=(kc == NCH - 1))
                    nc.scalar.activation(out=h1_all[:, mm, :], in_=h1_ps[:],
                                         func=AF.Gelu,
                                         bias=bias_t['b1'][:, mm:mm + 1],
                                         scale=1.0)
                m2b = tp9.tile([128, NCH, 2, N], BF16, tag='st9_m2b',
                               name='st9_m2b')
                for oc in range(NCH):
                    m2_ps = pp9.tile([128, 392], FP32, tag='acc392',
                                     name='m2ps')
                    for mm in range(MMCH):
                        nc.tensor.matmul(
                            m2_ps[:], W2T[mm][:, oc * 128:(oc + 1) * 128],
                            h1_all[:, mm, :],
                            start=(mm == 0), stop=(mm == MMCH - 1))
                    nc.scalar.activation(
                        out=m2b[:, oc, :, :],
                        in_=m2_ps[:].rearrange('p (b n) -> p b n', b=2),
                        func=AF.Identity, bias=bias_t['b2'][:, oc:oc + 1])
                for bi, b in enumerate((bp, bp + 1)):
                    for i, (off, nsz) in enumerate(NCK):
                        x2r = tp9.tile([128, C], FP32, tag='st9_x2r',
                                       name='st9_x2r')
                        nc.sync.dma_start(out=x2r[:nsz],
                                          in_=x2_dram[b, off:off + nsz, :])
                        ot = tp9.tile([128, C], FP32, tag='st9_out',
                                      name='st9_out')
                        for oc in range(NCH):
                            mpt = pp9b.tile([128, 128], BF16, tag='tp128',
                                            name='st9_mpt')
                            nc.tensor.transpose(
                                mpt[:nsz, :], m2b[:, oc, bi, off:off + nsz],
                                id16)
                            nc.vector.tensor_add(
                                out=ot[:nsz, oc * 128:(oc + 1) * 128],
                                in0=mpt[:nsz, :],
                                in1=x2r[:nsz, oc * 128:(oc + 1) * 128])
                        nc.sync.dma_start(out=out_dram[b, off:off + nsz, :],
                                          in_=ot[:nsz])


def _conv_offset_sample(nc, tc, d, h, bp, qp, xnpad, dwdiag, E8, id32, id16,
                        bias_t, refy_t, refx_t, rc3_t, wyb, wxb, oeps_t,
                        gbt, sampled, tp3, sm, pp3, pp3b):
    """ST3 (conv) + ST4 (offset head -> 3x3 hat weights) + ST5 (stencil
    sample) for a batch pair (bp, bp+1).

    Bilinear grid-sample is exact as a 3x3 stencil here: |offset| < 1 px, so
    all 4 taps lie in the 3x3 neighborhood of each pixel, with weight
    w(dy,dx) = relu(1-|gy-row-dy|) * relu(1-|gx-col-dx|); the zeroed pad
    border reproduces the reference's out-of-bounds masking."""
    ocT = {b: tp3.tile([128, 2, C], BF16, tag='st3_ocT', name='st3_ocT', bufs=2)
           for b in (bp, bp + 1)}
    for oc in range(NCH):
        for bi, b in enumerate((bp, bp + 1)):
            cv_ps = pp3.tile([128, 256], FP32, tag='acc392c', name='st3_ps')
            for t in range(9):
                ky, kx = divmod(t, 3)
                d0 = b * QPW + 16 * ky + kx
                nc.tensor.matmul(
                    cv_ps[:],
                    dwdiag[:, (t * NCH + oc) * 128:(t * NCH + oc + 1) * 128],
                    qp[oc][:, d0:d0 + 256],
                    start=(t == 0), stop=(t == 8))
            cvb = tp3.tile([128, N], BF16, tag='st3_cvb', name='st3_cvb')
            base = cv_ps[:, 0:1]
            inap = bass.AP(tensor=base.tensor, offset=base.offset,
                           ap=[base.ap[0], [16, 14], [1, 14]])
            nc.scalar.activation(out=cvb[:], in_=inap, func=AF.Identity,
                                 bias=bias_t['dwb'][:, oc:oc + 1])
            for i, (off, nsz) in enumerate(NCK):
                pt = pp3b.tile([128, 128], BF16, tag='tp128c', name='st3_tp')
                nc.tensor.transpose(pt[:nsz, :], cvb[:, off:off + nsz], id16)
                nc.vector.tensor_copy(
                    out=ocT[b][:nsz, i, oc * 128:(oc + 1) * 128], in_=pt[:nsz, :])

    W72 = {}
    for bi, b in enumerate((bp, bp + 1)):
        W72[b] = tp3.tile([G, 9 * N], BF16, tag='w72', name='st4_w72', bufs=2)
        for i, (off, nsz) in enumerate(NCK):
            sl = ocT[b][:nsz, i, :]
            st8 = sm.tile([128, G, 6], FP32, tag='off_st', name='off_st')
            mv8 = sm.tile([128, G, 2], FP32, tag='off_mv', name='off_mv')
            for g in range(G):
                nc.vector.bn_stats(out=st8[:nsz, g, :],
                                   in_=sl[:, g * CG:(g + 1) * CG])
                nc.vector.bn_aggr(out=mv8[:nsz, g, :], in_=st8[:nsz, g, :])
            std8 = sm.tile([128, G], FP32, tag='off_std', name='off_std')
            nc.scalar.activation(out=std8[:nsz], in_=mv8[:nsz, :, 1],
                                 func=AF.Sqrt, bias=oeps_t[:nsz], scale=1.0)
            rec8 = sm.tile([128, G], FP32, tag='off_rec', name='off_rec')
            nc.vector.reciprocal(out=rec8[:nsz], in_=std8[:nsz])
            og = tp3.tile([128, C], BF16, tag='off_og', name='off_og', bufs=2)
            ogv = og[:nsz].rearrange('p (g c) -> p g c', g=G)
            nc.vector.tensor_tensor(out=ogv,
                                    in0=sl.rearrange('p (g c) -> p g c', g=G),
                                    in1=_free_bcast(mv8[:nsz, :, 0], CG),
                                    op=ALU.subtract)
            nc.vector.tensor_tensor(out=ogv, in0=ogv,
                                    in1=_free_bcast(rec8[:nsz], CG), op=ALU.mult)
            if not h['offln_trivial']:
                gt, bt = gbt['offln']
                nc.vector.tensor_mul(out=og[:nsz], in0=og[:nsz], in1=gt[:nsz])
                nc.vector.tensor_add(out=og[:nsz], in0=og[:nsz], in1=bt[:nsz])
            nc.scalar.activation(out=og[:nsz], in_=og[:nsz], func=AF.Gelu)
            oyx = sm.tile([128, 16], FP32, tag='off_oyx', name='off_oyx')
            tpm = tp3.tile([128, C], BF16, tag='off_tpm', name='off_tpm', bufs=2)
            nc.vector.tensor_mul(out=tpm[:nsz], in0=og[:nsz], in1=wyb[:nsz])
            nc.vector.tensor_reduce(
                out=oyx[:nsz, 0:G],
                in_=tpm[:nsz].rearrange('p (g c) -> p g c', g=G),
                axis=mybir.AxisListType.X, op=ALU.add)
            nc.vector.tensor_mul(out=tpm[:nsz], in0=og[:nsz], in1=wxb[:nsz])
            nc.vector.tensor_reduce(
                out=oyx[:nsz, G:16],
                in_=tpm[:nsz].rearrange('p (g c) -> p g c', g=G),
                axis=mybir.AxisListType.X, op=ALU.add)
            th = sm.tile([128, 16], FP32, tag='off_th', name='off_th')
            nc.scalar.activation(out=th[:nsz], in_=oyx[:nsz], func=AF.Tanh)
            gg = sm.tile([128, 16], FP32, tag='off_gg', name='off_gg')
            nc.vector.tensor_scalar(out=gg[:nsz, 0:G], in0=th[:nsz, 0:G],
                                    scalar1=6.5 / 14.0, scalar2=refy_t[i][:],
                                    op0=ALU.mult, op1=ALU.add)
            nc.vector.tensor_scalar(out=gg[:nsz, G:16], in0=th[:nsz, G:16],
                                    scalar1=6.5 / 14.0, scalar2=refx_t[i][:],
                                    op0=ALU.mult, op1=ALU.add)
            # hat weights: w[d] = relu(1 - |g - center_d|) = 1 - min(|.|, 1)
            hat = sm.tile([128, 3, 16], FP32, tag='off_hat', name='off_hat')
            ggb = gg[:nsz, 0:16]
            gg_bc = bass.AP(tensor=ggb.tensor, offset=ggb.offset,
                            ap=[ggb.ap[0], [0, 3], [1, 16]])
            tt = sm.tile([128, 3, 16], FP32, tag='off_tt', name='off_tt')
            nc.vector.tensor_tensor(out=tt[:nsz], in0=gg_bc,
                                    in1=rc3_t[i][:nsz], op=ALU.subtract)
            up = sm.tile([128, 3, 16], FP32, tag='off_up', name='off_up')
            nc.vector.tensor_scalar(out=up[:nsz], in0=tt[:nsz],
                                    scalar1=1.0, scalar2=None, op0=ALU.add)
            nc.vector.tensor_scalar(out=tt[:nsz], in0=tt[:nsz],
                                    scalar1=-1.0, scalar2=1.0,
                                    op0=ALU.mult, op1=ALU.add)
            nc.vector.tensor_tensor(out=hat[:nsz], in0=tt[:nsz],
                                    in1=up[:nsz], op=ALU.min)
            nc.vector.tensor_scalar_max(out=hat[:nsz], in0=hat[:nsz],
                                        scalar1=0.0)
            # 9 products wy[dy]*wx[dx] -> [nsz, (dy,dx,g)=72]
            w2 = sm.tile([128, 9, G], FP32, tag='off_w2', name='off_w2')
            hb = hat[:nsz, 0, 0:1]
            wy_bc = bass.AP(tensor=hb.tensor, offset=hb.offset,
                            ap=[hb.ap[0], [16, 3], [0, 3], [1, G]])
            wx_bc = bass.AP(tensor=hb.tensor, offset=hb.offset + G,
                            ap=[hb.ap[0], [0, 3], [16, 3], [1, G]])
            w2o = w2[:nsz, 0, 0:1]
            w2_ap = bass.AP(tensor=w2o.tensor, offset=w2o.offset,
                            ap=[w2o.ap[0], [3 * G, 3], [G, 3], [1, G]])
            nc.vector.tensor_tensor(out=w2_ap, in0=wy_bc, in1=wx_bc,
                                    op=ALU.mult)
            for s in range(9):
                ptw2 = pp3.tile([G, 128], FP32, tag='tp8w', name='off_ptw2')
                nc.tensor.transpose(ptw2[:, :nsz], w2[:nsz, s, :],
                                    id32[:nsz, :nsz])
                nc.vector.tensor_copy(
                    out=W72[b][:, s * N + off:s * N + off + nsz],
                    in_=ptw2[:, :nsz])
        if DEBUG:
            nc.sync.dma_start(out=d['dbg_w72'][b], in_=W72[b][:])

    # ST5: expand group weights to channels (PE) and apply the 3x3 stencil
    for bi, b in enumerate((bp, bp + 1)):
        for j in range(NCH):
            w9e = tp3.tile([128, 9 * N], BF16, tag='st5_w9e', name='st5_w9e',
                           bufs=2)
            for g5 in range(5):
                # 2 stencil slots per PSUM tile: 392 f32 = 1568B fits one
                # 2KB bank (a single matmul must not straddle a bank)
                wexp = pp3.tile([128, 2 * N], FP32, tag='wexp',
                                name='st5_wexp', bufs=2)
                ns = 2 if g5 < 4 else 1
                for k in range(ns):
                    s = 2 * g5 + k
                    nc.tensor.matmul(wexp[:, k * N:(k + 1) * N],
                                     E8[:, j * 128:(j + 1) * 128],
                                     W72[b][:, s * N:(s + 1) * N],
                                     start=True, stop=True)
                nc.scalar.activation(
                    out=w9e[:, 2 * g5 * N:(2 * g5 + ns) * N],
                    in_=wexp[:, :ns * N], func=AF.Identity)
            tmp9 = tp3.tile([128, 9, N], BF16, tag='st5_tmp9', name='st5_tmp9',
                            bufs=2)
            for dy in range(3):
                for dx in range(3):
                    s = dy * 3 + dx
                    ib = xnpad[j][:, b * QPW + 17 + 16 * (dy - 1) + (dx - 1):
                                  b * QPW + 17 + 16 * (dy - 1) + dx]
                    img_ap = bass.AP(tensor=ib.tensor, offset=ib.offset,
                                     ap=[ib.ap[0], [16, 14], [1, 14]])
                    nc.vector.tensor_tensor(out=tmp9[:, s, :], in0=img_ap,
                                            in1=w9e[:, s * N:(s + 1) * N],
                                            op=ALU.mult)
            tb = tmp9[:, 0, 0:1]
            red_in = bass.AP(tensor=tb.tensor, offset=tb.offset,
                             ap=[tb.ap[0], [1, N], [N, 9]])
            with nc.allow_low_precision(reason='bilinear: <=4 nonzero terms, '
                                               'weights sum to <=1'):
                nc.vector.tensor_reduce(out=sampled[j][:, b, :], in_=red_in,
                                        axis=mybir.AxisListType.X, op=ALU.add)


def build_nc(h):
    from concourse import bacc
    nc = bacc.Bacc(None, target_bir_lowering=False, debug=False)
    d = {}

    def din(name, shape, dt):
        d[name] = nc.declare_dram_parameter(name, list(shape), dt, isOutput=False)

    din('x_shard', (BL, N, C), FP32)
    out_dram = nc.declare_dram_parameter('out', [BL, N, C], FP32, isOutput=True)
    x2_dram = nc.dram_tensor('x2_scratch', [BL, N, C], FP32)
    if DEBUG:
        d['dbg_xnp'] = nc.declare_dram_parameter(
            'dbg_xnp', [NCH, 128, BL * QPW], BF16, isOutput=True)
        d['dbg_smp'] = nc.declare_dram_parameter(
            'dbg_smp', [NCH, 128, BL, N], BF16, isOutput=True)
        d['dbg_w72'] = nc.declare_dram_parameter(
            'dbg_w72', [BL, G, 9 * N], BF16, isOutput=True)

    din('WqT', (C, C), BF16); din('WkT', (C, C), BF16)
    din('WvT', (C, C), BF16); din('WoT', (C, C), BF16)
    din('W1T', (C, MLPD), BF16); din('W2T', (MLPD, C), BF16)
    din('bq', (128, NCH), FP32); din('bk', (128, NCH), FP32)
    din('bo', (128, NCH), FP32); din('b1', (128, MMCH), FP32)
    din('b2', (128, NCH), FP32)
    din('dwdiag', (9, NCH, 128, 128), BF16); din('dwb', (128, NCH), FP32)
    din('E8', (G, C), BF16)
    din('refy', (N,), FP32); din('refx', (N,), FP32)
    din('rc3', (N, 48), FP32)
    din('wyv', (C,), BF16); din('wxv', (C,), BF16)
    din('id32', (128, 128), FP32); din('id16', (128, 128), BF16)
    if not h['ln1_trivial']:
        din('ln1_g', (C,), FP32); din('ln1_b', (C,), FP32)
    if not h['ln2_trivial']:
        din('ln2_g', (C,), FP32); din('ln2_b', (C,), FP32)
    if not h['offln_trivial']:
        din('offln_g', (C,), FP32); din('offln_b', (C,), FP32)
    if not h['bv_trivial']:
        din('bv', (128, C), FP32)

    with tile.TileContext(nc) as tc:
        emit(nc, tc, d, out_dram, x2_dram, h)
    nc.compile()
    return nc


_DECLARED = {'WqT', 'WkT', 'WvT', 'WoT', 'W1T', 'W2T', 'bq', 'bk', 'bo',
             'b1', 'b2', 'dwdiag', 'dwb', 'E8', 'refy', 'refx',
             'wyv', 'wxv', 'id32', 'id16', 'rc3'}

_CACHE = {}


def kernel(**inputs):
    h = build_host_consts(inputs)
    if 'nc' not in _CACHE:
        _CACHE['nc'] = build_nc(h)
    nc = _CACHE['nc']

    declared = set(_DECLARED)
    for nm in ('ln1', 'ln2', 'offln'):
        if not h[nm + '_trivial']:
            declared |= {nm + '_g', nm + '_b'}
    if not h['bv_trivial']:
        declared.add('bv')
    shared = {k: v for k, v in h.items()
              if k in declared and isinstance(v, np.ndarray)}

    x = _f32(inputs['x'])
    in_maps = []
    for c in range(NCORES):
        m = dict(shared)
        m['x_shard'] = np.ascontiguousarray(x[c * BL:(c + 1) * BL])
        in_maps.append(m)
    res = run_bass_kernel_spmd(nc, in_maps, list(range(NCORES)))
    outs = [res.results[c]['out'] for c in range(NCORES)]
    return np.concatenate(outs, axis=0).astype(np.float32)



# revision 31
# speedup vs baseline: 1.8209x; 1.0720x over previous
"""Trainium2 Bass kernel for a DAT-style transformer block (sparse_attention).

kernel(**inputs) takes FULL unsharded inputs (B=64), shards the batch across
8 NeuronCores (8 per core, pure data parallel — no collectives), runs one SPMD
Bass/Tile program, returns the FULL [64, 196, 768] float32 output.

Per-core pipeline (8 local batches):
  ST1 LN1 + PE transposes -> xnTb (bf16) + xnpad (bf16, zero-padded 16x16 grid)
  ST2 q = Wq@xnT + bq  -> qp, padded [16,16] spatial layout (bf16)
  ST3 depthwise 3x3 conv via 9 accumulating diag-matmuls; transpose -> ocT
  ST4 offset head: group-LN -> GELU -> proj -> tanh -> pixel coords ->
      3x3 separable hat weights per (group, pixel) (exact bilinear: |offset|
      < 1 px keeps all 4 taps inside the 3x3 stencil; pad border = zeros)
  ST5 expand weights group->channel (PE matmul vs E8), 9 shifted multiplies
      on xnpad + reduce (DVE) -> sampled
  ST6 k = Wk@sampled + bk; vT = sampled^T @ Wv^T
  ST7 attention per head (S -> exp(+rowsum) -> norm -> P^T -> @v), o-proj,
      residual 1, LN2 (x2 spilled to DRAM), transposes -> xn2T
  ST9 MLP (24x fused h1 -> GELU -> accumulate m2), bias, transpose, residual 2
All matmuls bf16 with fp32 PSUM accumulation; stats/softmax/residuals fp32.
"""

import numpy as np
import ml_dtypes

import concourse.bass as bass
import concourse.mybir as mybir
import concourse.tile as tile
from concourse import library_config
from concourse.bass_utils import run_bass_kernel_spmd

FP32 = mybir.dt.float32
BF16 = mybir.dt.bfloat16
I16 = mybir.dt.int16
AF = mybir.ActivationFunctionType
ALU = mybir.AluOpType

B = 64
NCORES = 8
BL = 8
N = 196
C = 768
NCH = 6
HEADS = 12
HD = 64
G = 8
CG = 96
MLPD = 3072
MMCH = 24
HH = 14
NCK = [(0, 128), (128, 68)]
MCK = [(0, 98), (98, 98)]
EPS = 1e-6
OFF_EPS = 1e-5
NPAD = 208
NTAP = 4
QPW = 290
DEBUG = False


def _f32(x):
    return np.ascontiguousarray(np.asarray(x), dtype=np.float32)


def _bf16(x):
    return np.ascontiguousarray(
        np.asarray(x, dtype=np.float32).astype(ml_dtypes.bfloat16))


def build_host_consts(inp):
    h = {}
    h['WqT'] = _bf16(np.asarray(inp['Wq'], np.float32).T)
    h['WkT'] = _bf16(np.asarray(inp['Wk'], np.float32).T)
    h['WvT'] = _bf16(np.asarray(inp['Wv'], np.float32).T)
    h['WoT'] = _bf16(np.asarray(inp['Wo'], np.float32).T)
    h['W1T'] = _bf16(np.asarray(inp['W1'], np.float32).T)
    h['W2T'] = _bf16(np.asarray(inp['W2'], np.float32).T)

    h['bq'] = _f32(np.asarray(inp['bq']).reshape(NCH, 128).T)
    h['bk'] = _f32(np.asarray(inp['bk']).reshape(NCH, 128).T)
    h['bo'] = _f32(np.asarray(inp['bo']).reshape(NCH, 128).T)
    h['b1'] = _f32(np.asarray(inp['b1']).reshape(MMCH, 128).T)
    h['b2'] = _f32(np.asarray(inp['b2']).reshape(NCH, 128).T)

    dw = np.asarray(inp['off_dw_w'], np.float32).reshape(CG, 9)
    dwg = np.tile(dw, (G, 1))
    diag = np.zeros((9, NCH, 128, 128), np.float32)
    for t in range(9):
        for cc in range(NCH):
            np.fill_diagonal(diag[t, cc], dwg[cc * 128:(cc + 1) * 128, t])
    h['dwdiag'] = _bf16(diag)
    h['dwb'] = _f32(np.tile(np.asarray(inp['off_dw_b'], np.float32), G)
                    .reshape(NCH, 128).T)

    e8 = np.zeros((G, C), np.float32)
    for c in range(C):
        e8[c // CG, c] = 1.0
    h['E8'] = _bf16(e8)

    ii = np.arange(HH, dtype=np.float32)
    h['refy'] = _f32(np.repeat((ii + 0.5) * 13.0 / 14.0, HH))
    h['refx'] = _f32(np.tile((ii + 0.5) * 13.0 / 14.0, HH))
    # rc3[n, d*16+g] = row(n)+(d-1) for g<8 else col(n)+(d-1): hat centers
    rowi = np.repeat(ii, HH)
    colj = np.tile(ii, HH)
    rc3 = np.zeros((N, 48), np.float32)
    for dd in range(3):
        rc3[:, dd * 16:dd * 16 + 8] = (rowi + dd - 1)[:, None]
        rc3[:, dd * 16 + 8:dd * 16 + 16] = (colj + dd - 1)[:, None]
    h['rc3'] = _f32(rc3)

    pw = np.asarray(inp['off_proj_w'], np.float32)
    h['wyv'] = _bf16(np.tile(pw[0], G))
    h['wxv'] = _bf16(np.tile(pw[1], G))

    h['id32'] = _f32(np.eye(128, dtype=np.float32))
    h['id16'] = _bf16(np.eye(128, dtype=np.float32))

    for nm, gk, bk_ in (('ln1', 'ln1_g', 'ln1_b'), ('ln2', 'ln2_g', 'ln2_b')):
        g = np.asarray(inp[gk], np.float32)
        bb = np.asarray(inp[bk_], np.float32)
        h[nm + '_trivial'] = bool(np.all(g == 1.0) and np.all(bb == 0.0))
        h[nm + '_g'] = _f32(g)
        h[nm + '_b'] = _f32(bb)
    og = np.tile(np.asarray(inp['off_ln_g'], np.float32), G)
    ob = np.tile(np.asarray(inp['off_ln_b'], np.float32), G)
    h['offln_trivial'] = bool(np.all(og == 1.0) and np.all(ob == 0.0))
    h['offln_g'] = _f32(og)
    h['offln_b'] = _f32(ob)
    bv = np.asarray(inp['bv'], np.float32)
    h['bv_trivial'] = bool(np.all(bv == 0.0))
    h['bv'] = _f32(np.tile(bv.reshape(1, C), (128, 1)))
    return h


def _free_bcast(t_ap, inner):
    """View [P, F] AP as [P, F, inner] with a stride-0 inner dim."""
    return bass.AP(tensor=t_ap.tensor, offset=t_ap.offset,
                   ap=list(t_ap.ap) + [[0, inner]])


def _dram_bcast(src_ap, rows):
    return bass.AP(tensor=src_ap.tensor, offset=src_ap.offset,
                   ap=[[0, rows]] + list(src_ap.ap))


def emit(nc, tc, d, out_dram, x2_dram, h):
    x_in = d['x_shard']

    with (
        tc.tile_pool(name='cw', bufs=1) as cw,
        tc.tile_pool(name='p_xn2T', bufs=1) as p_xn2T,
        tc.tile_pool(name='p_sm', bufs=4) as sm,
    ):
        # ---- always-resident constants --------------------------------
        WoT = [cw.tile([128, C], BF16, tag=f'wo{k}', name=f'wo{k}')
               for k in range(NCH)]
        for k in range(NCH):
            nc.sync.dma_start(out=WoT[k][:], in_=d['WoT'][k * 128:(k + 1) * 128, :])
        E8 = cw.tile([G, C], BF16, tag='e8', name='e8')
        nc.sync.dma_start(out=E8[:], in_=d['E8'][:])
        id32 = cw.tile([128, 128], FP32, tag='id32', name='id32')
        id16 = cw.tile([128, 128], BF16, tag='id16', name='id16')
        nc.sync.dma_start(out=id32[:], in_=d['id32'][:])
        nc.sync.dma_start(out=id16[:], in_=d['id16'][:])
        bias_t = {}
        for nm, cols in (('bq', NCH), ('bk', NCH), ('bo', NCH), ('b1', MMCH),
                         ('b2', NCH), ('dwb', NCH)):
            bias_t[nm] = cw.tile([128, cols], FP32, tag='bias_' + nm,
                                 name='bias_' + nm)
            nc.sync.dma_start(out=bias_t[nm][:], in_=d[nm][:])
        refy_t, refx_t, rc3_t = [], [], []
        for i, (off, nsz) in enumerate(NCK):
            for nm, lst in (('refy', refy_t), ('refx', refx_t)):
                tt = cw.tile([nsz, 1], FP32, tag=f'{nm}{i}', name=f'{nm}{i}')
                nc.sync.dma_start(
                    out=tt[:],
                    in_=d[nm][off:off + nsz].rearrange('(n one) -> n one',
                                                       one=1))
                lst.append(tt)
            rt = cw.tile([nsz, 3, 16], FP32, tag=f'rc3{i}', name=f'rc3{i}')
            nc.sync.dma_start(
                out=rt[:],
                in_=d['rc3'][off:off + nsz].rearrange('n (d g) -> n d g', d=3))
            rc3_t.append(rt)
        wyb = cw.tile([128, C], BF16, tag='wyb', name='wyb')
        wxb = cw.tile([128, C], BF16, tag='wxb', name='wxb')
        nc.sync.dma_start(out=wyb[:], in_=_dram_bcast(d['wyv'][:], 128))
        nc.sync.dma_start(out=wxb[:], in_=_dram_bcast(d['wxv'][:], 128))
        eps_t = cw.tile([128, 1], FP32, tag='eps', name='eps')
        nc.vector.memset(eps_t[:], EPS)
        oeps_t = cw.tile([128, 1], FP32, tag='oeps', name='oeps')
        nc.vector.memset(oeps_t[:], OFF_EPS)
        gbt = {}
        for nm in ('ln1', 'ln2', 'offln'):
            if not h[nm + '_trivial']:
                g_ = cw.tile([128, C], FP32, tag=nm + 'g', name=nm + 'g')
                b_ = cw.tile([128, C], FP32, tag=nm + 'b', name=nm + 'b')
                nc.sync.dma_start(out=g_[:], in_=_dram_bcast(d[nm + '_g'][:], 128))
                nc.sync.dma_start(out=b_[:], in_=_dram_bcast(d[nm + '_b'][:], 128))
                gbt[nm] = (g_, b_)
        bv_t = None
        if not h['bv_trivial']:
            bv_t = cw.tile([128, C], FP32, tag='bvt', name='bvt')
            nc.sync.dma_start(out=bv_t[:], in_=d['bv'][:])

        def ln_norm(xf, nsz, out_ap, gbk, tmp_pool):
            st = sm.tile([128, 3, 6], FP32, tag='ln_st', name='ln_st')
            for s in range(3):
                nc.vector.bn_stats(out=st[:nsz, s, :],
                                   in_=xf[:nsz, s * 256:(s + 1) * 256])
            mv = sm.tile([128, 2], FP32, tag='ln_mv', name='ln_mv')
            nc.vector.bn_aggr(out=mv[:nsz], in_=st[:nsz])
            std = sm.tile([128, 1], FP32, tag='ln_std', name='ln_std')
            nc.scalar.activation(out=std[:nsz], in_=mv[:nsz, 1:2], func=AF.Sqrt,
                                 bias=eps_t[:nsz], scale=1.0)
            rstd = sm.tile([128, 1], FP32, tag='ln_rstd', name='ln_rstd')
            nc.vector.reciprocal(out=rstd[:nsz], in_=std[:nsz])
            nmr = sm.tile([128, 1], FP32, tag='ln_nmr', name='ln_nmr')
            nc.vector.tensor_scalar(out=nmr[:nsz], in0=mv[:nsz, 0:1],
                                    scalar1=rstd[:nsz], scalar2=-1.0,
                                    op0=ALU.mult, op1=ALU.mult)
            if gbk is None:
                nc.scalar.activation(out=out_ap, in_=xf[:nsz], func=AF.Identity,
                                     bias=nmr[:nsz], scale=rstd[:nsz])
            else:
                gt, bt = gbk
                tmp = tmp_pool.tile([128, C], FP32, tag='ln_tmp', name='ln_tmp')
                nc.scalar.activation(out=tmp[:nsz], in_=xf[:nsz], func=AF.Identity,
                                     bias=nmr[:nsz], scale=rstd[:nsz])
                nc.vector.tensor_mul(out=tmp[:nsz], in0=tmp[:nsz], in1=gt[:nsz])
                nc.vector.tensor_add(out=out_ap, in0=tmp[:nsz], in1=bt[:nsz])

        xn2T = [p_xn2T.tile([128, BL, N], BF16, tag=f'x2T{k}', name=f'x2T{k}')
                for k in range(NCH)]

        with tc.tile_pool(name='p_qp', bufs=1) as p_qp:
            q_pl = [p_qp.tile([128, BL, N], BF16, tag=f'qpl{k}', name=f'qpl{k}')
                    for k in range(NCH)]
            with tc.tile_pool(name='p_smp', bufs=1) as p_smp:
                sampled = [p_smp.tile([128, BL, N], BF16, tag=f'smp{k}',
                                      name=f'smp{k}') for k in range(NCH)]
                with tc.tile_pool(name='p_qpad', bufs=1) as p_qpad:
                    qp = [p_qpad.tile([128, BL * QPW], BF16, tag=f'qp{k}',
                                      name=f'qp{k}') for k in range(NCH)]
                    xnpad = [p_qpad.tile([128, BL * QPW], BF16, tag=f'xnp{k}',
                                         name=f'xnp{k}') for k in range(NCH)]
                    for k in range(NCH):
                        nc.gpsimd.memset(qp[k][:], 0.0)
                        nc.gpsimd.memset(xnpad[k][:], 0.0)
                    with (
                        tc.tile_pool(name='p_st12', bufs=1) as p12,
                        tc.tile_pool(name='t_st12', bufs=3) as tp1,
                        tc.tile_pool(name='tb_st12', bufs=2) as tb1,
                        tc.tile_pool(name='ps_st12', bufs=3, space='PSUM') as pp1,
                        tc.tile_pool(name='ps_st12b', bufs=2, space='PSUM') as pp1b,
                    ):
                        WqT = [p12.tile([128, C], BF16, tag=f'wq{k}', name=f'wq{k}')
                               for k in range(NCH)]
                        for k in range(NCH):
                            nc.sync.dma_start(out=WqT[k][:],
                                              in_=d['WqT'][k * 128:(k + 1) * 128, :])
                        xnTb = [p12.tile([128, BL, N], BF16, tag=f'xnTb{k}',
                                         name=f'xnTb{k}') for k in range(NCH)]

                        for b in range(BL):
                            for i, (off, nsz) in enumerate(NCK):
                                xf = tb1.tile([128, C], FP32, tag='st1_x', name='st1_x')
                                nc.sync.dma_start(out=xf[:nsz],
                                                  in_=x_in[b, off:off + nsz, :])
                                xn = tb1.tile([128, C], FP32, tag='st1_xn', name='st1_xn')
                                ln_norm(xf, nsz, xn[:nsz], gbt.get('ln1'), tb1)
                                for cc in range(NCH):
                                    pt = pp1.tile([128, 128], FP32, tag='tp128',
                                                  name='st1_ps')
                                    nc.tensor.transpose(
                                        pt[:, :nsz], xn[:nsz, cc * 128:(cc + 1) * 128],
                                        id32[:nsz, :nsz])
                                    nc.scalar.activation(
                                        out=xnTb[cc][:, b, off:off + nsz],
                                        in_=pt[:, :nsz], func=AF.Identity)
                        # pad xnTb into the zeroed 16x16 grid (all 8 batches,
                        # one scalar copy per channel chunk)
                        for cc in range(NCH):
                            sb = xnTb[cc][:, 0, 0:1]
                            src = bass.AP(tensor=sb.tensor, offset=sb.offset,
                                          ap=[sb.ap[0], [N, BL], [14, 14], [1, 14]])
                            db = xnpad[cc][:, 17:18]
                            dst = bass.AP(tensor=db.tensor, offset=db.offset,
                                          ap=[db.ap[0], [QPW, BL], [16, 14], [1, 14]])
                            nc.scalar.activation(out=dst, in_=src, func=AF.Identity)

                        for bp in range(0, BL, 2):
                            for oc in range(NCH):
                                q_ps = pp1b.tile([128, 392], FP32, tag='acc392',
                                                 name='st2_ps')
                                for kc in range(NCH):
                                    nc.tensor.matmul(
                                        q_ps[:], WqT[kc][:, oc * 128:(oc + 1) * 128],
                                        xnTb[kc][:, bp:bp + 2, :],
                                        start=(kc == 0), stop=(kc == NCH - 1))
                                for bi, b in enumerate((bp, bp + 1)):
                                    base = qp[oc][:, b * QPW + 17:b * QPW + 18]
                                    outap = bass.AP(tensor=base.tensor,
                                                    offset=base.offset,
                                                    ap=[base.ap[0], [16, 14], [1, 14]])
                                    nc.scalar.activation(
                                        out=outap,
                                        in_=q_ps[:, bi * N:(bi + 1) * N],
                                        func=AF.Identity,
                                        bias=bias_t['bq'][:, oc:oc + 1])
                                nc.vector.tensor_copy(
                                    out=q_pl[oc][:, bp:bp + 2, :],
                                    in_=q_ps[:].rearrange('p (b n) -> p b n', b=2))

                    # ============ ST3..ST5 (pair loop), then ST6, ST7 ===========
                    with (
                        tc.tile_pool(name='p_cs', bufs=1) as pcs,
                        tc.tile_pool(name='t_cs', bufs=3) as tp3,
                        tc.tile_pool(name='ps_cs', bufs=2, space='PSUM') as pp3,
                        tc.tile_pool(name='ps_cs2', bufs=2, space='PSUM') as pp3b,
                    ):
                        dwdiag = pcs.tile([128, 9 * NCH * 128], BF16, tag='dwdiag',
                                          name='dwdiag')
                        nc.sync.dma_start(
                            out=dwdiag[:].rearrange('p (t c m) -> p t c m',
                                                    t=9, c=NCH),
                            in_=d['dwdiag'][:].rearrange('t c p m -> p t c m'))

                        for bp in range(0, BL, 2):
                            _conv_offset_sample(
                                nc, tc, d, h, bp, qp, xnpad, dwdiag, E8, id32,
                                id16, bias_t, refy_t, refx_t, rc3_t,
                                wyb, wxb, oeps_t, gbt,
                                sampled, tp3, sm, pp3, pp3b)
                        if DEBUG:
                            for j in range(NCH):
                                nc.sync.dma_start(out=d['dbg_xnp'][j],
                                                  in_=xnpad[j][:])
                                nc.sync.dma_start(out=d['dbg_smp'][j],
                                                  in_=sampled[j][:])

                with (
                    tc.tile_pool(name='p_kv', bufs=1) as pkv,
                    tc.tile_pool(name='t_67', bufs=3) as tp7,
                    tc.tile_pool(name='tb_67', bufs=2) as tb7,
                    tc.tile_pool(name='ps_67', bufs=2, space='PSUM') as pp7,
                    tc.tile_pool(name='ps_67b', bufs=2, space='PSUM') as pp7b,
                ):
                    WkT = [pkv.tile([128, C], BF16, tag=f'wk{k}', name=f'wk{k}')
                           for k in range(NCH)]
                    WvT = [pkv.tile([128, C], BF16, tag=f'wv{k}', name=f'wv{k}')
                           for k in range(NCH)]
                    for k in range(NCH):
                        nc.sync.dma_start(out=WkT[k][:],
                                          in_=d['WkT'][k * 128:(k + 1) * 128, :])
                        nc.sync.dma_start(out=WvT[k][:],
                                          in_=d['WvT'][k * 128:(k + 1) * 128, :])
                    k_all = [pkv.tile([128, BL, N], BF16, tag=f'kk{k}',
                                      name=f'kk{k}') for k in range(NCH)]
                    vT_all = pkv.tile([128, BL, 2, C], BF16, tag='vT', name='vT')

                    # ---------------- ST6 --------------------------------
                    for bp in range(0, BL, 2):
                        for oc in range(NCH):
                            k_ps = pp7.tile([128, 392], FP32, tag='acc392',
                                            name='st6_kps')
                            for kc in range(NCH):
                                nc.tensor.matmul(
                                    k_ps[:], WkT[kc][:, oc * 128:(oc + 1) * 128],
                                    sampled[kc][:, bp:bp + 2, :],
                                    start=(kc == 0), stop=(kc == NCH - 1))
                            nc.scalar.activation(
                                out=k_all[oc][:, bp:bp + 2, :],
                                in_=k_ps[:].rearrange('p (b n) -> p b n', b=2),
                                func=AF.Identity,
                                bias=bias_t['bk'][:, oc:oc + 1])
                    for b in range(BL):
                        for i, (off, nsz) in enumerate(NCK):
                            for half in range(2):
                                v_ps = pp7.tile([128, 384], FP32, tag='acc392',
                                                name='st6_vps')
                                for kc in range(NCH):
                                    nc.tensor.matmul(
                                        v_ps[:nsz],
                                        sampled[kc][:, b, off:off + nsz],
                                        WvT[kc][:, half * 384:(half + 1) * 384],
                                        start=(kc == 0), stop=(kc == NCH - 1))
                                dst = vT_all[:nsz, b, i,
                                             half * 384:(half + 1) * 384]
                                if bv_t is None:
                                    nc.scalar.activation(out=dst,
                                                         in_=v_ps[:nsz],
                                                         func=AF.Identity)
                                else:
                                    nc.vector.tensor_add(
                                        out=dst, in0=v_ps[:nsz],
                                        in1=bv_t[:nsz,
                                                 half * 384:(half + 1) * 384])

                    # ---------------- ST7 --------------------------------
                    for bp in range(0, BL, 2):
                        aop = tp7.tile([128, NCH, 2, N], BF16, tag='st7_ao',
                                       name='st7_ao')
                        for bi, b in enumerate((bp, bp + 1)):
                            for hp in range(NCH):
                                o_ps = pp7b.tile([128, N], FP32, tag='st7_ops',
                                                 name='st7_ops')
                                for hh in range(2):
                                    hd = hp * 2 + hh
                                    p0 = (hd % 2) * 64
                                    PT = [tp7.tile([128, N], BF16, tag='st7_pt',
                                                   name='st7_pt')
                                          for _ in range(2)]
                                    for mi, (moff, msz) in enumerate(MCK):
                                        s_ps = pp7b.tile([98, N], FP32,
                                                         tag='st7_sps',
                                                         name='st7_sps')
                                        nc.tensor.matmul(
                                            s_ps[:],
                                            q_pl[hp][p0:p0 + 64, b,
                                                     moff:moff + msz],
                                            k_all[hp][p0:p0 + 64, b, :],
                                            start=True, stop=True)
                                        expP = tp7.tile([98, N], BF16,
                                                        tag='st7_exp',
                                                        name='st7_exp')
                                        ssum = sm.tile([98, 1], FP32,
                                                       tag='st7_ssum',
                                                       name='st7_ssum')
                                        nc.scalar.activation(
                                            out=expP[:], in_=s_ps[:],
                                            func=AF.Exp, scale=0.125,
                                            accum_out=ssum[:])
                                        srec = sm.tile([98, 1], FP32,
                                                       tag='st7_srec',
                                                       name='st7_srec')
                                        nc.vector.reciprocal(out=srec[:],
                                                             in_=ssum[:])
                                        nc.vector.tensor_scalar(
                                            out=expP[:], in0=expP[:],
                                            scalar1=srec[:], scalar2=None,
                                            op0=ALU.mult)
                                        for ni, (noff, nsz) in enumerate(NCK):
                                            ptp = pp7.tile([128, 98], BF16,
                                                           tag='tp128',
                                                           name='st7_ptp')
                                            nc.tensor.transpose(
                                                ptp[:nsz, :],
                                                expP[:, noff:noff + nsz],
                                                id16[:98, :98])
                                            nc.vector.tensor_copy(
                                                out=PT[ni][:nsz,
                                                           moff:moff + msz],
                                                in_=ptp[:nsz, :])
                                    for ni, (noff, nsz) in enumerate(NCK):
                                        nc.tensor.matmul(
                                            o_ps[p0:p0 + 64, :],
                                            vT_all[:nsz, b, ni,
                                                   hd * 64:(hd + 1) * 64],
                                            PT[ni][:nsz, :],
                                            start=(ni == 0), stop=(ni == 1))
                                nc.vector.tensor_copy(out=aop[:, hp, bi, :],
                                                      in_=o_ps[:])
                        ybf = tp7.tile([128, NCH, 2, N], BF16, tag='st7_ybf',
                                       name='st7_ybf')
                        for oc in range(NCH):
                            y_ps = pp7.tile([128, 392], FP32, tag='acc392',
                                            name='st7_yps')
                            for kc in range(NCH):
                                nc.tensor.matmul(
                                    y_ps[:], WoT[kc][:, oc * 128:(oc + 1) * 128],
                                    aop[:, kc, :, :],
                                    start=(kc == 0), stop=(kc == NCH - 1))
                            nc.scalar.activation(
                                out=ybf[:, oc, :, :],
                                in_=y_ps[:].rearrange('p (b n) -> p b n', b=2),
                                func=AF.Identity,
                                bias=bias_t['bo'][:, oc:oc + 1])
                        for bi, b in enumerate((bp, bp + 1)):
                            for i, (off, nsz) in enumerate(NCK):
                                xo = tb7.tile([128, C], FP32, tag='st7_xo',
                                              name='st7_xo')
                                nc.sync.dma_start(out=xo[:nsz],
                                                  in_=x_in[b, off:off + nsz, :])
                                x2 = tb7.tile([128, C], FP32, tag='st7_x2',
                                              name='st7_x2')
                                for oc in range(NCH):
                                    ypt = pp7.tile([128, 128], BF16, tag='tp128',
                                                   name='st7_ypt')
                                    nc.tensor.transpose(
                                        ypt[:nsz, :],
                                        ybf[:, oc, bi, off:off + nsz], id16)
                                    nc.vector.tensor_add(
                                        out=x2[:nsz, oc * 128:(oc + 1) * 128],
                                        in0=ypt[:nsz, :],
                                        in1=xo[:nsz, oc * 128:(oc + 1) * 128])
                                nc.sync.dma_start(
                                    out=x2_dram[b, off:off + nsz, :],
                                    in_=x2[:nsz])
                                xn2 = tb7.tile([128, C], FP32, tag='st7_xn2',
                                               name='st7_xn2')
                                ln_norm(x2, nsz, xn2[:nsz], gbt.get('ln2'), tb7)
                                for cc in range(NCH):
                                    pt = pp7.tile([128, 128], FP32, tag='tp128',
                                                  name='st7_tps')
                                    nc.tensor.transpose(
                                        pt[:, :nsz],
                                        xn2[:nsz, cc * 128:(cc + 1) * 128],
                                        id32[:nsz, :nsz])
                                    nc.vector.tensor_copy(
                                        out=xn2T[cc][:, b, off:off + nsz],
                                        in_=pt[:, :nsz])

        # ===================== ST9: MLP + residual2 ======================
        with (
            tc.tile_pool(name='p_mlp', bufs=1) as pm,
            tc.tile_pool(name='t_mlp', bufs=2) as tp9,
            tc.tile_pool(name='ps_mlp', bufs=2, space='PSUM') as pp9,
            tc.tile_pool(name='ps_mlp2', bufs=2, space='PSUM') as pp9b,
        ):
            W1T = [pm.tile([128, MLPD], BF16, tag=f'w1_{k}', name=f'w1_{k}')
                   for k in range(NCH)]
            for k in range(NCH):
                nc.sync.dma_start(out=W1T[k][:],
                                  in_=d['W1T'][k * 128:(k + 1) * 128, :])
            W2T = [pm.tile([128, C], BF16, tag=f'w2_{m}', name=f'w2_{m}')
                   for m in range(MMCH)]
            for m in range(MMCH):
                nc.sync.dma_start(out=W2T[m][:],
                                  in_=d['W2T'][m * 128:(m + 1) * 128, :])

            for bp in range(0, BL, 2):
                h1_all = tp9.tile([128, MMCH, 392], BF16, tag='h1_all',
                                  name='h1_all')
                for mm in range(MMCH):
                    h1_ps = pp9.tile([128, 392], FP32, tag='h1ps', name='h1ps')
                    for kc in range(NCH):
                        nc.tensor.matmul(
                            h1_ps[:], W1T[kc][:, mm * 128:(mm + 1) * 128],
                            xn2T[kc][:, bp:bp + 2, :],
                            start=(kc == 0), stop=(kc == NCH - 1))
                    nc.scalar.activation(out=h1_all[:, mm, :], in_=h1_ps[:],
                                         func=AF.Gelu,
                                         bias=bias_t['b1'][:, mm:mm + 1],
                                         scale=1.0)
                m2b = tp9.tile([128, NCH, 2, N], BF16, tag='st9_m2b',
                               name='st9_m2b')
                for oc in range(NCH):
                    m2_ps = pp9.tile([128, 392], FP32, tag='acc392',
                                     name='m2ps')
                    for mm in range(MMCH):
                        nc.tensor.matmul(
                            m2_ps[:], W2T[mm][:, oc * 128:(oc + 1) * 128],
                            h1_all[:, mm, :],
                            start=(mm == 0), stop=(mm == MMCH - 1))
                    nc.scalar.activation(
                        out=m2b[:, oc, :, :],
                        in_=m2_ps[:].rearrange('p (b n) -> p b n', b=2),
                        func=AF.Identity, bias=bias_t['b2'][:, oc:oc + 1])
                for bi, b in enumerate((bp, bp + 1)):
                    for i, (off, nsz) in enumerate(NCK):
                        x2r = tp9.tile([128, C], FP32, tag='st9_x2r',
                                       name='st9_x2r')
                        nc.sync.dma_start(out=x2r[:nsz],
                                          in_=x2_dram[b, off:off + nsz, :])
                        ot = tp9.tile([128, C], FP32, tag='st9_out',
                                      name='st9_out')
                        for oc in range(NCH):
                            mpt = pp9b.tile([128, 128], BF16, tag='tp128',
                                            name='st9_mpt')
                            nc.tensor.transpose(
                                mpt[:nsz, :], m2b[:, oc, bi, off:off + nsz],
                                id16)
                            nc.vector.tensor_add(
                                out=ot[:nsz, oc * 128:(oc + 1) * 128],
                                in0=mpt[:nsz, :],
                                in1=x2r[:nsz, oc * 128:(oc + 1) * 128])
                        nc.sync.dma_start(out=out_dram[b, off:off + nsz, :],
                                          in_=ot[:nsz])


def _conv_offset_sample(nc, tc, d, h, bp, qp, xnpad, dwdiag, E8, id32, id16,
                        bias_t, refy_t, refx_t, rc3_t, wyb, wxb, oeps_t,
                        gbt, sampled, tp3, sm, pp3, pp3b):
    """ST3 (conv) + ST4 (offset head -> 3x3 hat weights) + ST5 (stencil
    sample) for a batch pair (bp, bp+1).

    Bilinear grid-sample is exact as a 3x3 stencil here: |offset| < 1 px, so
    all 4 taps lie in the 3x3 neighborhood of each pixel, with weight
    w(dy,dx) = relu(1-|gy-row-dy|) * relu(1-|gx-col-dx|); the zeroed pad
    border reproduces the reference's out-of-bounds masking."""
    ocT = {b: tp3.tile([128, 2, C], BF16, tag='st3_ocT', name='st3_ocT', bufs=2)
           for b in (bp, bp + 1)}
    for oc in range(NCH):
        for bi, b in enumerate((bp, bp + 1)):
            cv_ps = pp3.tile([128, 256], FP32, tag='acc392c', name='st3_ps')
            for t in range(9):
                ky, kx = divmod(t, 3)
                d0 = b * QPW + 16 * ky + kx
                nc.tensor.matmul(
                    cv_ps[:],
                    dwdiag[:, (t * NCH + oc) * 128:(t * NCH + oc + 1) * 128],
                    qp[oc][:, d0:d0 + 256],
                    start=(t == 0), stop=(t == 8))
            cvb = tp3.tile([128, N], BF16, tag='st3_cvb', name='st3_cvb')
            base = cv_ps[:, 0:1]
            inap = bass.AP(tensor=base.tensor, offset=base.offset,
                           ap=[base.ap[0], [16, 14], [1, 14]])
            nc.scalar.activation(out=cvb[:], in_=inap, func=AF.Identity,
                                 bias=bias_t['dwb'][:, oc:oc + 1])
            for i, (off, nsz) in enumerate(NCK):
                pt = pp3b.tile([128, 128], BF16, tag='tp128c', name='st3_tp')
                nc.tensor.transpose(pt[:nsz, :], cvb[:, off:off + nsz], id16)
                nc.vector.tensor_copy(
                    out=ocT[b][:nsz, i, oc * 128:(oc + 1) * 128], in_=pt[:nsz, :])

    W72 = {}
    for bi, b in enumerate((bp, bp + 1)):
        W72[b] = tp3.tile([G, 9 * N], BF16, tag='w72', name='st4_w72', bufs=2)
        for i, (off, nsz) in enumerate(NCK):
            sl = ocT[b][:nsz, i, :]
            st8 = sm.tile([128, G, 6], FP32, tag='off_st', name='off_st')
            mv8 = sm.tile([128, G, 2], FP32, tag='off_mv', name='off_mv')
            for g in range(G):
                nc.vector.bn_stats(out=st8[:nsz, g, :],
                                   in_=sl[:, g * CG:(g + 1) * CG])
                nc.vector.bn_aggr(out=mv8[:nsz, g, :], in_=st8[:nsz, g, :])
            std8 = sm.tile([128, G], FP32, tag='off_std', name='off_std')
            nc.scalar.activation(out=std8[:nsz], in_=mv8[:nsz, :, 1],
                                 func=AF.Sqrt, bias=oeps_t[:nsz], scale=1.0)
            rec8 = sm.tile([128, G], FP32, tag='off_rec', name='off_rec')
            nc.vector.reciprocal(out=rec8[:nsz], in_=std8[:nsz])
            og = tp3.tile([128, C], BF16, tag='off_og', name='off_og', bufs=2)
            ogv = og[:nsz].rearrange('p (g c) -> p g c', g=G)
            nc.vector.tensor_tensor(out=ogv,
                                    in0=sl.rearrange('p (g c) -> p g c', g=G),
                                    in1=_free_bcast(mv8[:nsz, :, 0], CG),
                                    op=ALU.subtract)
            nc.vector.tensor_tensor(out=ogv, in0=ogv,
                                    in1=_free_bcast(rec8[:nsz], CG), op=ALU.mult)
            if not h['offln_trivial']:
                gt, bt = gbt['offln']
                nc.vector.tensor_mul(out=og[:nsz], in0=og[:nsz], in1=gt[:nsz])
                nc.vector.tensor_add(out=og[:nsz], in0=og[:nsz], in1=bt[:nsz])
            nc.scalar.activation(out=og[:nsz], in_=og[:nsz], func=AF.Gelu)
            oyx = sm.tile([128, 16], FP32, tag='off_oyx', name='off_oyx')
            tpm = tp3.tile([128, C], BF16, tag='off_tpm', name='off_tpm', bufs=2)
            nc.vector.tensor_mul(out=tpm[:nsz], in0=og[:nsz], in1=wyb[:nsz])
            nc.vector.tensor_reduce(
                out=oyx[:nsz, 0:G],
                in_=tpm[:nsz].rearrange('p (g c) -> p g c', g=G),
                axis=mybir.AxisListType.X, op=ALU.add)
            nc.vector.tensor_mul(out=tpm[:nsz], in0=og[:nsz], in1=wxb[:nsz])
            nc.vector.tensor_reduce(
                out=oyx[:nsz, G:16],
                in_=tpm[:nsz].rearrange('p (g c) -> p g c', g=G),
                axis=mybir.AxisListType.X, op=ALU.add)
            th = sm.tile([128, 16], FP32, tag='off_th', name='off_th')
            nc.scalar.activation(out=th[:nsz], in_=oyx[:nsz], func=AF.Tanh)
            gg = sm.tile([128, 16], FP32, tag='off_gg', name='off_gg')
            nc.vector.tensor_scalar(out=gg[:nsz, 0:G], in0=th[:nsz, 0:G],
                                    scalar1=6.5 / 14.0, scalar2=refy_t[i][:],
                                    op0=ALU.mult, op1=ALU.add)
            nc.vector.tensor_scalar(out=gg[:nsz, G:16], in0=th[:nsz, G:16],
                                    scalar1=6.5 / 14.0, scalar2=refx_t[i][:],
                                    op0=ALU.mult, op1=ALU.add)
            # hat weights: w[d] = relu(1 - |g - center_d|) = 1 - min(|.|, 1)
            hat = sm.tile([128, 3, 16], FP32, tag='off_hat', name='off_hat')
            ggb = gg[:nsz, 0:16]
            gg_bc = bass.AP(tensor=ggb.tensor, offset=ggb.offset,
                            ap=[ggb.ap[0], [0, 3], [1, 16]])
            tt = sm.tile([128, 3, 16], FP32, tag='off_tt', name='off_tt')
            nc.vector.tensor_tensor(out=tt[:nsz], in0=gg_bc,
                                    in1=rc3_t[i][:nsz], op=ALU.subtract)
            up = sm.tile([128, 3, 16], FP32, tag='off_up', name='off_up')
            nc.vector.tensor_scalar(out=up[:nsz], in0=tt[:nsz],
                                    scalar1=1.0, scalar2=None, op0=ALU.add)
            nc.vector.tensor_scalar(out=tt[:nsz], in0=tt[:nsz],
                                    scalar1=-1.0, scalar2=1.0,
                                    op0=ALU.mult, op1=ALU.add)
            nc.vector.tensor_tensor(out=hat[:nsz], in0=tt[:nsz],
                                    in1=up[:nsz], op=ALU.min)
            nc.vector.tensor_scalar_max(out=hat[:nsz], in0=hat[:nsz],
                                        scalar1=0.0)
            # 9 products wy[dy]*wx[dx] -> [nsz, (dy,dx,g)=72] in bf16
            w2 = sm.tile([128, 9, G], BF16, tag='off_w2', name='off_w2')
            hb = hat[:nsz, 0, 0:1]
            wy_bc = bass.AP(tensor=hb.tensor, offset=hb.offset,
                            ap=[hb.ap[0], [16, 3], [0, 3], [1, G]])
            wx_bc = bass.AP(tensor=hb.tensor, offset=hb.offset + G,
                            ap=[hb.ap[0], [0, 3], [16, 3], [1, G]])
            w2o = w2[:nsz, 0, 0:1]
            w2_ap = bass.AP(tensor=w2o.tensor, offset=w2o.offset,
                            ap=[w2o.ap[0], [3 * G, 3], [G, 3], [1, G]])
            nc.vector.tensor_tensor(out=w2_ap, in0=wy_bc, in1=wx_bc,
                                    op=ALU.mult)
            ptw9 = pp3.tile([G, 9 * 128], BF16, tag='ptw9', name='off_ptw9',
                            bufs=1)
            for s in range(9):
                nc.tensor.transpose(ptw9[:, s * 128:s * 128 + nsz],
                                    w2[:nsz, s, :], id16[:nsz, :nsz])
            pb = ptw9[:, 0:1]
            src = bass.AP(tensor=pb.tensor, offset=pb.offset,
                          ap=[pb.ap[0], [128, 9], [1, nsz]])
            wb_ = W72[b][:, off:off + 1]
            dst = bass.AP(tensor=wb_.tensor, offset=wb_.offset,
                          ap=[wb_.ap[0], [N, 9], [1, nsz]])
            nc.vector.tensor_copy(out=dst, in_=src)
        if DEBUG:
            nc.sync.dma_start(out=d['dbg_w72'][b], in_=W72[b][:])

    # ST5: expand group weights to channels (PE) and apply the 3x3 stencil,
    # both batches of the pair at once; mults/adds split DVE / GpSimd
    for j in range(NCH):
        w9e = tp3.tile([128, 9, 2 * N], BF16, tag='st5_w9e', name='st5_w9e',
                       bufs=2)
        for s in range(9):
            # one PSUM bank per slot pair: 392 f32 = 1568B (a single matmul
            # must not straddle a 2KB bank)
            wexp = pp3.tile([128, 2 * N], FP32, tag='wexp',
                            name='st5_wexp', bufs=2)
            for bi, b in enumerate((bp, bp + 1)):
                nc.tensor.matmul(wexp[:, bi * N:(bi + 1) * N],
                                 E8[:, j * 128:(j + 1) * 128],
                                 W72[b][:, s * N:(s + 1) * N],
                                 start=True, stop=True)
            nc.scalar.activation(out=w9e[:, s, :], in_=wexp[:],
                                 func=AF.Identity)

        def img_ap(s):
            dy, dx = divmod(s, 3)
            o0 = bp * QPW + 17 + 16 * (dy - 1) + (dx - 1)
            ib = xnpad[j][:, o0:o0 + 1]
            return bass.AP(tensor=ib.tensor, offset=ib.offset,
                           ap=[ib.ap[0], [QPW, 2], [16, 14], [1, 14]])

        tmpA = tp3.tile([128, 5, 2 * N], BF16, tag='st5_tA', name='st5_tA',
                        bufs=2)
        tmpB = tp3.tile([128, 4, 2 * N], BF16, tag='st5_tB', name='st5_tB',
                        bufs=2)
        for k in range(5):
            nc.vector.tensor_tensor(out=tmpA[:, k, :], in0=img_ap(k),
                                    in1=w9e[:, k, :], op=ALU.mult)
        for k in range(4):
            nc.gpsimd.tensor_tensor(out=tmpB[:, k, :], in0=img_ap(5 + k),
                                    in1=w9e[:, 5 + k, :], op=ALU.mult)
        aa = tp3.tile([128, 2 * N], BF16, tag='st5_aa', name='st5_aa', bufs=2)
        ab = tp3.tile([128, 2 * N], BF16, tag='st5_ab', name='st5_ab', bufs=2)
        cc = tp3.tile([128, 2 * N], BF16, tag='st5_cc', name='st5_cc', bufs=2)
        cd = tp3.tile([128, 2 * N], BF16, tag='st5_cd', name='st5_cd', bufs=2)
        nc.vector.tensor_add(out=aa[:], in0=tmpA[:, 0, :], in1=tmpA[:, 1, :])
        nc.vector.tensor_add(out=ab[:], in0=tmpA[:, 2, :], in1=tmpA[:, 3, :])
        nc.vector.tensor_add(out=aa[:], in0=aa[:], in1=ab[:])
        nc.vector.tensor_add(out=aa[:], in0=aa[:], in1=tmpA[:, 4, :])
        nc.gpsimd.tensor_add(out=cc[:], in0=tmpB[:, 0, :], in1=tmpB[:, 1, :])
        nc.gpsimd.tensor_add(out=cd[:], in0=tmpB[:, 2, :], in1=tmpB[:, 3, :])
        nc.gpsimd.tensor_add(out=cc[:], in0=cc[:], in1=cd[:])
        nc.vector.tensor_add(
            out=sampled[j][:, bp:bp + 2, :].rearrange('p b n -> p (b n)'),
            in0=aa[:], in1=cc[:])


def build_nc(h):
    from concourse import bacc
    nc = bacc.Bacc(None, target_bir_lowering=False, debug=False)
    d = {}

    def din(name, shape, dt):
        d[name] = nc.declare_dram_parameter(name, list(shape), dt, isOutput=False)

    din('x_shard', (BL, N, C), FP32)
    out_dram = nc.declare_dram_parameter('out', [BL, N, C], FP32, isOutput=True)
    x2_dram = nc.dram_tensor('x2_scratch', [BL, N, C], FP32)
    if DEBUG:
        d['dbg_xnp'] = nc.declare_dram_parameter(
            'dbg_xnp', [NCH, 128, BL * QPW], BF16, isOutput=True)
        d['dbg_smp'] = nc.declare_dram_parameter(
            'dbg_smp', [NCH, 128, BL, N], BF16, isOutput=True)
        d['dbg_w72'] = nc.declare_dram_parameter(
            'dbg_w72', [BL, G, 9 * N], BF16, isOutput=True)

    din('WqT', (C, C), BF16); din('WkT', (C, C), BF16)
    din('WvT', (C, C), BF16); din('WoT', (C, C), BF16)
    din('W1T', (C, MLPD), BF16); din('W2T', (MLPD, C), BF16)
    din('bq', (128, NCH), FP32); din('bk', (128, NCH), FP32)
    din('bo', (128, NCH), FP32); din('b1', (128, MMCH), FP32)
    din('b2', (128, NCH), FP32)
    din('dwdiag', (9, NCH, 128, 128), BF16); din('dwb', (128, NCH), FP32)
    din('E8', (G, C), BF16)
    din('refy', (N,), FP32); din('refx', (N,), FP32)
    din('rc3', (N, 48), FP32)
    din('wyv', (C,), BF16); din('wxv', (C,), BF16)
    din('id32', (128, 128), FP32); din('id16', (128, 128), BF16)
    if not h['ln1_trivial']:
        din('ln1_g', (C,), FP32); din('ln1_b', (C,), FP32)
    if not h['ln2_trivial']:
        din('ln2_g', (C,), FP32); din('ln2_b', (C,), FP32)
    if not h['offln_trivial']:
        din('offln_g', (C,), FP32); din('offln_b', (C,), FP32)
    if not h['bv_trivial']:
        din('bv', (128, C), FP32)

    with tile.TileContext(nc) as tc:
        emit(nc, tc, d, out_dram, x2_dram, h)
    nc.compile()
    return nc


_DECLARED = {'WqT', 'WkT', 'WvT', 'WoT', 'W1T', 'W2T', 'bq', 'bk', 'bo',
             'b1', 'b2', 'dwdiag', 'dwb', 'E8', 'refy', 'refx',
             'wyv', 'wxv', 'id32', 'id16', 'rc3'}

_CACHE = {}


def kernel(**inputs):
    h = build_host_consts(inputs)
    if 'nc' not in _CACHE:
        _CACHE['nc'] = build_nc(h)
    nc = _CACHE['nc']

    declared = set(_DECLARED)
    for nm in ('ln1', 'ln2', 'offln'):
        if not h[nm + '_trivial']:
            declared |= {nm + '_g', nm + '_b'}
    if not h['bv_trivial']:
        declared.add('bv')
    shared = {k: v for k, v in h.items()
              if k in declared and isinstance(v, np.ndarray)}

    x = _f32(inputs['x'])
    in_maps = []
    for c in range(NCORES):
        m = dict(shared)
        m['x_shard'] = np.ascontiguousarray(x[c * BL:(c + 1) * BL])
        in_maps.append(m)
    res = run_bass_kernel_spmd(nc, in_maps, list(range(NCORES)))
    outs = [res.results[c]['out'] for c in range(NCORES)]
    return np.concatenate(outs, axis=0).astype(np.float32)

